# revision 17
# baseline (speedup 1.0000x reference)
"""Trainium2 Bass kernel for a 2-layer GAT (PyG semantics, eval mode).

SPMD over 8 NeuronCores, dst-sorted edge partitioning:
 - conv aggregation + softmax denominators fused into one-hot matmuls
   (psum += eq.T @ rhs) per 128-edge tile.
 - Per-edge gathers via batched Q7 dma_gather (1024 descs/call, the ucode
   ring limit) with cost-optimal element sizes: conv1 h rows 512B, conv1
   a_d 16B, conv2 rows 86B, conv2 a_d2 4B.
 - Source-row gather calls are stream-packed (cross node-tile boundaries,
   per-slot reads fragmented); a_d gathers are slot-windowed (in-tile
   indices, narrow table deps) and prefetched into persistent buffers so
   they overlap phase 0 (conv1) / conv1 compute (conv2).
 - conv2 uses weighted one-hots (fused is_equal*exp) and a constant-1
   table column: no per-edge rhs assembly, denominator from the same
   matmul.
 - h = x @ W1ext computed fully redundantly per core (c-major head
   layout); h2 AllGather in 4 chunks interleaved with conv1 compute.
"""
import sys

sys.path.insert(0, "/opt/trn_rl_repo")

import numpy as np

import concourse.bacc as bacc
import concourse.bass as bass
import concourse.mybir as mybir
import concourse.tile as tile
from concourse.bass_utils import run_bass_kernel_spmd

P = 128
N_DEV = 8
N_NODES = 50000
F_IN, HID, N_CLS, HEADS = 128, 16, 40, 8
NEG_SLOPE = 0.2
HALF = 32768                 # int16 index limit for dma_gather

N_TILES = 392
N_PAD = N_TILES * P          # 50176
TPD = N_TILES // N_DEV       # 49
NPD = TPD * P                # 6272

C1 = HEADS * HID             # 128 (h block, c-major: col c*8+h)
D1 = C1 + HEADS              # 136 rhs cols conv1: [h*ex | ex]
R1 = 256                     # conv1 table row elems (fp16, 512B stride)
RL = 128                     # conv2 table / ad_win row elems (256B stride)
D2 = N_CLS + 3               # 43 conv2 row: [h2lin(40)|a_s2|a_d2|one]
CMP = 44                     # compact collective row (43 used)
W2C = N_CLS + 2              # 42 = [W2 | w_as2 | w_ad2]

CT = 8                       # tiles per gather call (1024 descs: ring limit)
CB = 7                       # node tiles per post-processing chunk
NCOLL = 4                    # collective chunks
SLC = 8                      # src-idx calls per sidx load chunk

fp32 = mybir.dt.float32
fp16 = mybir.dt.float16
i16 = mybir.dt.int16

_CACHE = {}


def _patched_dma_gather():
    import inspect, textwrap
    src = inspect.getsource(bass.BassGpSimd.dma_gather)
    old = """        assert (
            elem_size_bytes > 0 and elem_size_bytes % 256 == 0
        )  # transpose restriction"""
    assert old in src, "bass dma_gather source changed; fall back to 256B elems"
    src = src.replace(old, """        assert elem_size_bytes > 0
        if transpose:
            assert elem_size_bytes % 256 == 0""")
    g = dict(bass.__dict__)
    exec(compile(textwrap.dedent(src), "<patched_dma_gather>", "exec"), g)
    return g["dma_gather"]


_PG = _patched_dma_gather()


def _wrap(idx_list):
    """int16 idx list -> [128, n/16] wrapped layout (j at [j%16, j//16],
    replicated across the 8 16-partition groups)."""
    n = len(idx_list)
    assert n % 16 == 0
    w = np.asarray(idx_list, dtype=np.int16).reshape(n // 16, 16).T
    return np.tile(w, (8, 1))


def _frags(cum, jj):
    """Split stream tiles [cum[jj], cum[jj+1]) at CT boundaries.
    Returns [(call, off_in_call, n, off_in_slot)]."""
    s0, s1 = int(cum[jj]), int(cum[jj + 1])
    res = []
    s = s0
    while s < s1:
        k = s // CT
        n = min((k + 1) * CT, s1) - s
        res.append((k, s - k * CT, n, s - s0))
        s += n
    return res


def _plan(edge_index):
    """Shared (cross-device) program plan + per-device data arrays."""
    src = np.concatenate([edge_index[0], np.arange(N_NODES, dtype=np.int32)])
    dst = np.concatenate([edge_index[1], np.arange(N_NODES, dtype=np.int32)])
    order = np.argsort(dst, kind="stable")
    src_s, dst_s = src[order], dst[order]
    bounds = np.searchsorted(dst_s, np.arange(N_TILES + 1) * P).astype(np.int64)

    lo_lists, hi_lists = [], []
    for j in range(N_TILES):
        s = src_s[bounds[j]:bounds[j + 1]]
        d = dst_s[bounds[j]:bounds[j + 1]]
        m = s < HALF
        lo_lists.append((s[m], d[m]))
        hi_lists.append((s[~m] - HALF, d[~m]))

    t1 = np.array([(len(lo_lists[j][0]) + P - 1) // P for j in range(N_TILES)])
    t2 = np.array([(len(hi_lists[j][0]) + P - 1) // P for j in range(N_TILES)])
    T1u = t1.reshape(N_DEV, TPD).max(axis=0)           # per-slot max
    T2u = t2.reshape(N_DEV, TPD).max(axis=0)
    Tu = T1u + T2u
    n_et = int(Tu.sum())

    lo_cum = np.concatenate([[0], np.cumsum(T1u)]).astype(int)
    hi_cum = np.concatenate([[0], np.cumsum(T2u)]).astype(int)
    d_cum = np.concatenate([[0], np.cumsum(Tu)]).astype(int)
    LOT, HIT = int(lo_cum[-1]), int(hi_cum[-1])
    n_lo_calls = (LOT + CT - 1) // CT
    n_hi_calls = (HIT + CT - 1) // CT

    plan = dict(T1u=T1u, T2u=T2u, Tu=Tu, n_et=n_et,
                lo_cum=lo_cum, hi_cum=hi_cum, d_cum=d_cum,
                LOT=LOT, HIT=HIT, n_lo_calls=n_lo_calls, n_hi_calls=n_hi_calls)

    dev = []
    for d in range(N_DEV):
        lo_stream = np.zeros(n_lo_calls * CT * P, np.int32)
        hi_stream = np.zeros(n_hi_calls * CT * P, np.int32)
        dit = np.zeros(n_et * P, np.int32)      # in-tile dst idx (0..127)
        dstl = np.full(n_et * P, 300.0, np.float32)
        for jj in range(TPD):
            j = d * TPD + jj
            sl, dl = lo_lists[j]
            sh, dh = hi_lists[j]
            lo_stream[lo_cum[jj] * P:lo_cum[jj] * P + len(sl)] = sl
            hi_stream[hi_cum[jj] * P:hi_cum[jj] * P + len(sh)] = sh
            base = d_cum[jj] * P
            dit[base:base + len(dl)] = dl - j * P
            dit[base + T1u[jj] * P:base + T1u[jj] * P + len(dh)] = dh - j * P
            dstl[base:base + len(dl)] = dl - j * P
            dstl[base + T1u[jj] * P:base + T1u[jj] * P + len(dh)] = dh - j * P
        sblocks = []
        for k in range(n_lo_calls):
            sblocks.append(_wrap(lo_stream[k * CT * P:(k + 1) * CT * P]))
        for k in range(n_hi_calls):
            sblocks.append(_wrap(hi_stream[k * CT * P:(k + 1) * CT * P]))
        src_widx = np.ascontiguousarray(
            np.concatenate(sblocks, axis=1)).astype(np.int16)
        dblocks = []
        for jj in range(TPD):
            T = int(Tu[jj])
            base = d_cum[jj] * P
            for c0 in range(0, T, CT):
                n = min(CT, T - c0)
                dblocks.append(_wrap(dit[base + c0 * P:base + (c0 + n) * P]))
        dst_widx = np.ascontiguousarray(
            np.concatenate(dblocks, axis=1)).astype(np.int16)
        dstl2 = np.ascontiguousarray(dstl.reshape(n_et, P).T)
        dev.append((src_widx, dst_widx, dstl2))
    return plan, dev


def _build(plan):
    T1u, T2u, Tu = plan["T1u"], plan["T2u"], plan["Tu"]
    n_et = plan["n_et"]
    lo_cum, hi_cum, d_cum = plan["lo_cum"], plan["hi_cum"], plan["d_cum"]
    LOT, HIT = plan["LOT"], plan["HIT"]
    n_lo_calls, n_hi_calls = plan["n_lo_calls"], plan["n_hi_calls"]
    Tmax = int(Tu.max())

    lo_col = lambda k: k * CT * 8
    hi_col = lambda k: (n_lo_calls + k) * CT * 8
    SRC_COLS = (n_lo_calls + n_hi_calls) * CT * 8
    dw_col = {}
    c = 0
    for jj in range(TPD):
        T = int(Tu[jj])
        for c0 in range(0, T, CT):
            n = min(CT, T - c0)
            dw_col[(jj, c0)] = c
            c += n * 8
    DST_COLS = c

    cbnd = [0, 14, 28, 42, TPD]
    coll_after_chunk = {2: [0], 4: [1], 5: [2], 6: [3]}

    nc = bacc.Bacc("TRN2", target_bir_lowering=False, debug=False,
                   num_devices=N_DEV, num_swdge_queues=4)

    xT = nc.dram_tensor("xT", [P, N_PAD], fp16, kind="ExternalInput")
    xT_loc = nc.dram_tensor("xT_loc", [P, NPD], fp16, kind="ExternalInput")
    w1ext = nc.dram_tensor("w1ext", [P, D1 + HEADS], fp16, kind="ExternalInput")
    w2ext = nc.dram_tensor("w2ext", [HID, W2C], fp16, kind="ExternalInput")
    b1b = nc.dram_tensor("b1b", [P, HID], fp32, kind="ExternalInput")
    b2b = nc.dram_tensor("b2b", [P, N_CLS], fp32, kind="ExternalInput")
    ident = nc.dram_tensor("ident", [P, P], fp16, kind="ExternalInput")
    iota = nc.dram_tensor("iota", [P, P], fp16, kind="ExternalInput")
    swidx = nc.dram_tensor("swidx", [P, SRC_COLS], i16, kind="ExternalInput")
    dwidx = nc.dram_tensor("dwidx", [P, DST_COLS], i16, kind="ExternalInput")
    dstli = nc.dram_tensor("dstli", [P, n_et], fp32, kind="ExternalInput")
    out = nc.dram_tensor("out", [NPD, N_CLS], fp32, kind="ExternalOutput")

    h_lo_t = nc.dram_tensor("h_lo_t", [HALF, R1], fp16)
    h_hi_t = nc.dram_tensor("h_hi_t", [N_PAD - HALF, R1], fp16)
    ad_win = nc.dram_tensor("ad_win", [NPD, RL], fp16)
    h2_loc = nc.dram_tensor("h2_loc", [NPD, RL], fp16)
    h2_cmp_loc = nc.dram_tensor("h2_cmp_loc", [NPD, CMP], fp16)
    h2_gath = nc.dram_tensor("h2_gath", [N_PAD, CMP], fp16, addr_space="Shared")
    h2_ext = nc.dram_tensor("h2_ext", [N_PAD, RL], fp16)

    rg = [list(range(N_DEV))]
    AO = mybir.AluOpType
    AF = mybir.ActivationFunctionType
    AX = mybir.AxisListType.X
    _q = [0]

    def qn():
        _q[0] = (_q[0] + 1) % 4
        return _q[0]

    with tile.TileContext(nc) as tc:
        with (
            tc.tile_pool(name="const", bufs=1) as cpool,
            tc.tile_pool(name="glo", bufs=6) as glo,
            tc.tile_pool(name="ghi", bufs=6) as ghi,
            tc.tile_pool(name="sidx", bufs=2) as sxp,
            tc.tile_pool(name="work", bufs=3) as wp,
            tc.tile_pool(name="eqp", bufs=32) as eqp,
            tc.tile_pool(name="small", bufs=2) as sm,
            tc.tile_pool(name="psA", bufs=3, space="PSUM") as psA,
            tc.tile_pool(name="psB", bufs=3, space="PSUM") as psB,
            tc.tile_pool(name="psC", bufs=2, space="PSUM") as psC,
        ):
            # ---------------- constants
            w1_sb = cpool.tile([P, D1 + HEADS], fp16)
            nc.sync.dma_start(out=w1_sb[:], in_=w1ext[:])
            w2_sb = cpool.tile([HID, W2C], fp16)
            nc.sync.dma_start(out=w2_sb[:], in_=w2ext[:])
            b1_sb = cpool.tile([P, HID], fp32)
            nc.sync.dma_start(out=b1_sb[:], in_=b1b[:])
            b2_sb = cpool.tile([P, N_CLS], fp32)
            nc.sync.dma_start(out=b2_sb[:], in_=b2b[:])
            id_sb = cpool.tile([P, P], fp16)
            nc.sync.dma_start(out=id_sb[:], in_=ident[:])
            iota_sb = cpool.tile([P, P], fp16)
            nc.sync.dma_start(out=iota_sb[:], in_=iota[:])
            dwidx_sb = cpool.tile([P, DST_COLS], i16)
            nc.sync.dma_start(out=dwidx_sb[:], in_=dwidx[:])
            dstl_sb = cpool.tile([P, n_et], fp32)
            nc.sync.dma_start(out=dstl_sb[:], in_=dstli[:])
            ones_sb = cpool.tile([P, 1], fp16)
            nc.vector.memset(ones_sb[:], 1.0)
            h1all = cpool.tile([P, TPD * HID], fp16)
            ad1c = cpool.tile([P, n_et * HEADS], fp16)   # conv1 a_d per edge
            ad2c = cpool.tile([P, n_et * 2], fp16)       # conv2 a_d2 per edge

            # ---------------- ad_win: local a_d rows from xT_loc
            hst2 = cpool.tile([P, TPD * HEADS], fp16)
            NC2 = 7
            for cc in range(TPD // NC2):
                xc2 = wp.tile([P, NC2 * P], fp16, tag="xc")
                nc.sync.dma_start(
                    out=xc2[:], in_=xT_loc[:, cc * NC2 * P:(cc + 1) * NC2 * P])
                psd = psB.tile([P, NC2 * HEADS], fp32, tag="acc", space="PSUM")
                for k in range(NC2):
                    nc.tensor.matmul(out=psd[:, k * HEADS:(k + 1) * HEADS],
                                     lhsT=xc2[:, k * P:(k + 1) * P],
                                     rhs=w1_sb[:, D1:D1 + HEADS],
                                     start=True, stop=True)
                nc.scalar.copy(out=hst2[:, cc * NC2 * HEADS:(cc + 1) * NC2 * HEADS],
                               in_=psd[:])
            nc.sync.dma_start(
                out=ad_win[:, 0:HEADS].rearrange("(t p) d -> p t d", p=P),
                in_=hst2[:].rearrange("p (t d) -> p t d", d=HEADS))

            # conv1 a_d prefetch calls: emitted interleaved with phase 0
            ad1_calls = [(jj, c0) for jj in range(TPD)
                         for c0 in range(0, int(Tu[jj]), CT)]
            ad1_pos = [0]

            def emit_ad1(k):
                for (jj, c0) in ad1_calls[ad1_pos[0]:ad1_pos[0] + k]:
                    n = min(CT, int(Tu[jj]) - c0)
                    dc = dw_col[(jj, c0)]
                    _PG(nc.gpsimd,
                        out_ap=ad1c[:].rearrange("p (t d) -> p t d", d=HEADS)[
                            :, int(d_cum[jj]) + c0:int(d_cum[jj]) + c0 + n, :],
                        in_ap=ad_win[jj * P:(jj + 1) * P, 0:HEADS],
                        idxs_ap=dwidx_sb[:, dc:dc + n * 8],
                        num_idxs=n * P, num_idxs_reg=n * P,
                        elem_size=HEADS, elem_step=RL, queue_num=qn())
                ad1_pos[0] += k

            # ---------------- phase 0: full-redundant h table (c-major rows)
            NCHUNK = 8
            for cc in range(N_TILES // NCHUNK):
                xc = wp.tile([P, NCHUNK * P], fp16, tag="xc")
                nc.sync.dma_start(
                    out=xc[:], in_=xT[:, cc * NCHUNK * P:(cc + 1) * NCHUNK * P])
                hst = wp.tile([P, NCHUNK * R1], fp16, tag="hst")
                for gi, (g0, gn) in enumerate(((0, 3), (3, 3), (6, 2))):
                    psh = psA.tile([P, 3 * (D1 + HEADS)], fp32, tag="big",
                                   space="PSUM")
                    for k in range(gn):
                        nc.tensor.matmul(
                            out=psh[:, k * (D1 + HEADS):(k + 1) * (D1 + HEADS)],
                            lhsT=xc[:, (g0 + k) * P:(g0 + k + 1) * P],
                            rhs=w1_sb[:], start=True, stop=True)
                    dst_view = hst[:].rearrange("p (k d) -> p k d", d=R1)[
                        :, g0:g0 + gn, 0:D1 + HEADS]
                    src_view = psh[:].rearrange("p (k d) -> p k d",
                                                d=D1 + HEADS)[:, 0:gn, :]
                    if gi % 2 == 0:
                        nc.scalar.copy(out=dst_view, in_=src_view)
                    else:
                        nc.vector.tensor_copy(out=dst_view, in_=src_view)
                r0 = cc * NCHUNK * P
                tgt = (h_lo_t[r0:r0 + NCHUNK * P, :] if r0 < HALF
                       else h_hi_t[r0 - HALF:r0 - HALF + NCHUNK * P, :])
                nc.sync.dma_start(
                    out=tgt.rearrange("(k p) d -> p k d", p=P),
                    in_=hst[:].rearrange("p (k d) -> p k d", d=R1))
                emit_ad1(4)

            emit_ad1(len(ad1_calls) - ad1_pos[0])

            # ---- conv1 post: ELU(mean(agg/den) + b1) -> h1all, then h2 rows
            def post1(acc, jj0, nb):
                a_v = acc[:].rearrange("p (b d) -> p b d", d=D1)
                den = sm.tile([P, CB * HEADS], fp32, tag="den")
                nc.vector.tensor_scalar(
                    out=den[:].rearrange("p (b h) -> p b h", h=HEADS)[:, 0:nb, :],
                    in0=a_v[:, 0:nb, C1:D1], scalar1=1e-16, scalar2=None,
                    op0=AO.add)
                rec = sm.tile([P, CB * HEADS], fp32, tag="rec")
                nc.vector.reciprocal(out=rec[:, 0:nb * HEADS],
                                     in_=den[:, 0:nb * HEADS])
                nc.vector.tensor_scalar(out=rec[:, 0:nb * HEADS],
                                        in0=rec[:, 0:nb * HEADS],
                                        scalar1=1.0 / HEADS, scalar2=None,
                                        op0=AO.mult)
                tmp = sm.tile([P, CB * C1], fp32, tag="tmp")
                nc.vector.tensor_tensor(
                    out=tmp[:].rearrange("p (b c h) -> p b c h",
                                         c=HID, h=HEADS)[:, 0:nb],
                    in0=a_v[:, 0:nb, 0:C1].rearrange("p b (c h) -> p b c h",
                                                     h=HEADS),
                    in1=rec[:].rearrange("p (b h) -> p b h", h=HEADS)[:, 0:nb, :]
                        .unsqueeze(2).to_broadcast([P, nb, HID, HEADS]),
                    op=AO.mult)
                h1b = sm.tile([P, CB * HID], fp32, tag="h1b")
                nc.vector.tensor_reduce(
                    out=h1b[:].rearrange("p (b c) -> p b c", c=HID)[:, 0:nb, :],
                    in_=tmp[:].rearrange("p (b c h) -> p b c h",
                                         c=HID, h=HEADS)[:, 0:nb],
                    axis=AX, op=AO.add)
                nc.vector.tensor_tensor(
                    out=h1b[:].rearrange("p (b c) -> p b c", c=HID)[:, 0:nb, :],
                    in0=h1b[:].rearrange("p (b c) -> p b c", c=HID)[:, 0:nb, :],
                    in1=b1_sb[:].unsqueeze(1).to_broadcast([P, nb, HID]),
                    op=AO.add)
                xm = sm.tile([P, CB * HID], fp32, tag="xm")
                nc.vector.tensor_scalar(out=xm[:, 0:nb * HID],
                                        in0=h1b[:, 0:nb * HID],
                                        scalar1=0.0, scalar2=None, op0=AO.min)
                em = sm.tile([P, CB * HID], fp32, tag="em")
                nc.scalar.activation(out=em[:, 0:nb * HID], in_=xm[:, 0:nb * HID],
                                     func=AF.Exp)
                xp = sm.tile([P, CB * HID], fp32, tag="xp")
                nc.vector.tensor_scalar(out=xp[:, 0:nb * HID],
                                        in0=h1b[:, 0:nb * HID],
                                        scalar1=0.0, scalar2=None, op0=AO.max)
                h1f = sm.tile([P, CB * HID], fp32, tag="h1f")
                nc.vector.tensor_tensor(out=h1f[:, 0:nb * HID],
                                        in0=em[:, 0:nb * HID],
                                        in1=xp[:, 0:nb * HID], op=AO.add)
                nc.vector.tensor_scalar(out=h1all[:, jj0 * HID:(jj0 + nb) * HID],
                                        in0=h1f[:, 0:nb * HID],
                                        scalar1=-1.0, scalar2=None, op0=AO.add)
                for i in range(nb):
                    jj = jj0 + i
                    pst = psC.tile([HID, P], fp16, tag="tp", space="PSUM")
                    nc.tensor.transpose(out=pst[:],
                                        in_=h1all[:, jj * HID:(jj + 1) * HID],
                                        identity=id_sb[:])
                    h1T = sm.tile([HID, P], fp16, tag="h1T")
                    nc.scalar.copy(out=h1T[:], in_=pst[:])
                    psh2 = psC.tile([P, W2C], fp32, tag="tp", space="PSUM")
                    nc.tensor.matmul(out=psh2[:], lhsT=h1T[:], rhs=w2_sb[:],
                                     start=True, stop=True)
                    h2st = sm.tile([P, CMP], fp16, tag="h2st")
                    nc.scalar.copy(out=h2st[:, 0:W2C], in_=psh2[:])
                    nc.vector.tensor_copy(out=h2st[:, W2C:W2C + 1], in_=ones_sb[:])
                    nc.sync.dma_start(out=h2_loc[jj * P:(jj + 1) * P, 0:CMP],
                                      in_=h2st[:])
                    nc.sync.dma_start(out=h2_cmp_loc[jj * P:(jj + 1) * P, :],
                                      in_=h2st[:])
                # conv2 a_d2 gathers for these tiles (overlap conv1/collective)
                for i in range(nb):
                    jj = jj0 + i
                    T = int(Tu[jj])
                    for c0 in range(0, T, CT):
                        n = min(CT, T - c0)
                        dc = dw_col[(jj, c0)]
                        _PG(nc.gpsimd,
                            out_ap=ad2c[:].rearrange("p (t d) -> p t d", d=2)[
                                :, int(d_cum[jj]) + c0:int(d_cum[jj]) + c0 + n, :],
                            in_ap=h2_loc[jj * P:(jj + 1) * P,
                                         N_CLS + 1:N_CLS + 3],
                            idxs_ap=dwidx_sb[:, dc:dc + n * 8],
                            num_idxs=n * P, num_idxs_reg=n * P,
                            elem_size=2, elem_step=RL, queue_num=qn())

            # ---- conv2 post: log_softmax(agg/den + b2) -> out
            def post2(acc, jj0, nb):
                a_v = acc[:].rearrange("p (b d) -> p b d", d=D2)
                den = sm.tile([P, CB], fp32, tag="den2")
                nc.vector.tensor_scalar(
                    out=den[:].rearrange("p (b o) -> p b o", o=1)[:, 0:nb, :],
                    in0=a_v[:, 0:nb, D2 - 1:D2], scalar1=1e-16, scalar2=None,
                    op0=AO.add)
                rec = sm.tile([P, CB], fp32, tag="rec2")
                nc.vector.reciprocal(out=rec[:, 0:nb], in_=den[:, 0:nb])
                h2f = sm.tile([P, CB * N_CLS], fp32, tag="h2f")
                h2f_v = h2f[:].rearrange("p (b c) -> p b c", c=N_CLS)
                nc.vector.tensor_tensor(
                    out=h2f_v[:, 0:nb, :], in0=a_v[:, 0:nb, 0:N_CLS],
                    in1=rec[:].rearrange("p (b o) -> p b o", o=1)[:, 0:nb, :]
                        .to_broadcast([P, nb, N_CLS]),
                    op=AO.mult)
                nc.vector.tensor_tensor(
                    out=h2f_v[:, 0:nb, :], in0=h2f_v[:, 0:nb, :],
                    in1=b2_sb[:].unsqueeze(1).to_broadcast([P, nb, N_CLS]),
                    op=AO.add)
                nm = sm.tile([P, CB], fp32, tag="nm")
                nc.vector.tensor_reduce(
                    out=nm[:].rearrange("p (b o) -> p b o", o=1)[:, 0:nb, :],
                    in_=h2f_v[:, 0:nb, :], axis=AX, op=AO.max, negate=True)
                hs = sm.tile([P, CB * N_CLS], fp32, tag="hs")
                hs_v = hs[:].rearrange("p (b c) -> p b c", c=N_CLS)
                nc.vector.tensor_tensor(
                    out=hs_v[:, 0:nb, :], in0=h2f_v[:, 0:nb, :],
                    in1=nm[:].rearrange("p (b o) -> p b o", o=1)[:, 0:nb, :]
                        .to_broadcast([P, nb, N_CLS]),
                    op=AO.add)
                es = sm.tile([P, CB * N_CLS], fp32, tag="es")
                nc.scalar.activation(out=es[:, 0:nb * N_CLS],
                                     in_=hs[:, 0:nb * N_CLS], func=AF.Exp)
                ssum = sm.tile([P, CB], fp32, tag="ssum")
                nc.vector.tensor_reduce(
                    out=ssum[:].rearrange("p (b o) -> p b o", o=1)[:, 0:nb, :],
                    in_=es[:].rearrange("p (b c) -> p b c", c=N_CLS)[:, 0:nb, :],
                    axis=AX, op=AO.add)
                lg = sm.tile([P, CB], fp32, tag="lg")
                nc.scalar.activation(out=lg[:, 0:nb], in_=ssum[:, 0:nb],
                                     func=AF.Ln)
                ot = sm.tile([P, CB * N_CLS], fp32, tag="ot")
                nc.vector.tensor_tensor(
                    out=ot[:].rearrange("p (b c) -> p b c", c=N_CLS)[:, 0:nb, :],
                    in0=hs_v[:, 0:nb, :],
                    in1=lg[:].rearrange("p (b o) -> p b o", o=1)[:, 0:nb, :]
                        .to_broadcast([P, nb, N_CLS]),
                    op=AO.subtract)
                nc.sync.dma_start(
                    out=out[jj0 * P:(jj0 + nb) * P, :]
                        .rearrange("(b p) d -> p b d", p=P),
                    in_=ot[:].rearrange("p (b c) -> p b c", c=N_CLS)[:, 0:nb, :])

            def emit_coll(q):
                r0, r1 = cbnd[q] * P, cbnd[q + 1] * P
                nc.gpsimd.collective_compute(
                    "AllGather", AO.bypass, replica_groups=rg,
                    ins=[h2_cmp_loc[r0:r1, :].opt()],
                    outs=[h2_gath[r0 * N_DEV:r1 * N_DEV, :].opt()])

            def emit_expand(q):
                r0, r1 = cbnd[q] * P, cbnd[q + 1] * P
                nc.sync.dma_start(
                    out=h2_ext[:, 0:CMP]
                        .rearrange("(d r) c -> d r c", d=N_DEV)[:, r0:r1, :],
                    in_=h2_gath[r0 * N_DEV:r1 * N_DEV, :]
                        .rearrange("(d r) c -> d r c", d=N_DEV, r=r1 - r0))

            # ---------------- shared conv loop
            def conv_pass(conv):
                RW = R1 if conv == 1 else D2
                lo_tiles = {}
                hi_tiles = {}
                next_lo = next_hi = 0
                state = {"lo": [None, 0, -1], "hi": [None, 0, -1]}  # tile,c0,c1

                def load_sidx(st, col0):
                    col1 = min(col0 + SLC * CT * 8, SRC_COLS)
                    t = sxp.tile([P, SLC * CT * 8], i16, tag="sidx_" + st)
                    nc.sync.dma_start(out=t[:, 0:col1 - col0],
                                      in_=swidx[:, col0:col1])
                    state[st] = [t, col0, col1]

                def idx_view(st, c0, ncols):
                    t, s0, s1 = state[st]
                    if t is None or c0 < s0 or c0 + ncols > s1:
                        load_sidx(st, c0)
                        t, s0, s1 = state[st]
                    return t[:, c0 - s0:c0 - s0 + ncols]

                def emit_lo(k):
                    nonlocal next_lo
                    nt = min(CT, LOT - k * CT)
                    iv = idx_view("lo", lo_col(k), nt * 8)
                    t = glo.tile([P, CT * RW], fp16,
                                 tag="rlo" if conv == 1 else "rlo2")
                    v = t[:].rearrange("p (t d) -> p t d", d=RW)
                    if conv == 1:
                        _PG(nc.gpsimd, out_ap=v[:, 0:nt, :], in_ap=h_lo_t[:, :],
                            idxs_ap=iv, num_idxs=nt * P, num_idxs_reg=nt * P,
                            elem_size=R1, queue_num=qn())
                    else:
                        _PG(nc.gpsimd, out_ap=v[:, 0:nt, :],
                            in_ap=h2_ext[0:HALF, 0:D2],
                            idxs_ap=iv, num_idxs=nt * P, num_idxs_reg=nt * P,
                            elem_size=D2, elem_step=RL, queue_num=qn())
                    lo_tiles[k] = v
                    next_lo = k + 1

                def emit_hi(k):
                    nonlocal next_hi
                    nt = min(CT, HIT - k * CT)
                    iv = idx_view("hi", hi_col(k), nt * 8)
                    t = ghi.tile([P, CT * RW], fp16,
                                 tag="rhi" if conv == 1 else "rhi2")
                    v = t[:].rearrange("p (t d) -> p t d", d=RW)
                    if conv == 1:
                        _PG(nc.gpsimd, out_ap=v[:, 0:nt, :], in_ap=h_hi_t[:, :],
                            idxs_ap=iv, num_idxs=nt * P, num_idxs_reg=nt * P,
                            elem_size=R1, queue_num=qn())
                    else:
                        _PG(nc.gpsimd, out_ap=v[:, 0:nt, :],
                            in_ap=h2_ext[HALF:N_PAD, 0:D2],
                            idxs_ap=iv, num_idxs=nt * P, num_idxs_reg=nt * P,
                            elem_size=D2, elem_step=RL, queue_num=qn())
                    hi_tiles[k] = v
                    next_hi = k + 1

                acc = None
                jj0 = 0
                for jj in range(TPD):
                    T1, T2, T = int(T1u[jj]), int(T2u[jj]), int(Tu[jj])
                    lofr = _frags(lo_cum, jj)
                    hifr = _frags(hi_cum, jj)
                    while next_lo * CT < lo_cum[jj + 1]:
                        emit_lo(next_lo)
                    while next_hi * CT < hi_cum[jj + 1]:
                        emit_hi(next_hi)

                    if jj % CB == 0:
                        jj0 = jj
                        acc = sm.tile([P, CB * (D1 if conv == 1 else D2)], fp32,
                                      tag="acc1" if conv == 1 else "acc2")

                    db = int(d_cum[jj])
                    if conv == 1:
                        ad_v = ad1c[:].rearrange("p (t d) -> p t d", d=HEADS)
                        e_t = sm.tile([P, Tmax * HEADS], fp16, tag="e")
                        e_v = e_t[:].rearrange("p (t h) -> p t h", h=HEADS)
                        for (k, co, n, so) in lofr:
                            nc.vector.tensor_tensor(
                                out=e_v[:, so:so + n, :],
                                in0=lo_tiles[k][:, co:co + n, C1:C1 + HEADS],
                                in1=ad_v[:, db + so:db + so + n, :], op=AO.add)
                        for (k, co, n, so) in hifr:
                            nc.vector.tensor_tensor(
                                out=e_v[:, T1 + so:T1 + so + n, :],
                                in0=hi_tiles[k][:, co:co + n, C1:C1 + HEADS],
                                in1=ad_v[:, db + T1 + so:db + T1 + so + n, :],
                                op=AO.add)
                        e2_t = sm.tile([P, Tmax * HEADS], fp16, tag="e2")
                        nc.vector.scalar_tensor_tensor(
                            out=e2_t[:, 0:T * HEADS], in0=e_t[:, 0:T * HEADS],
                            scalar=NEG_SLOPE, in1=e_t[:, 0:T * HEADS],
                            op0=AO.mult, op1=AO.max)
                        ex_t = sm.tile([P, Tmax * HEADS], fp16, tag="ex")
                        nc.scalar.activation(out=ex_t[:, 0:T * HEADS],
                                             in_=e2_t[:, 0:T * HEADS],
                                             func=AF.Exp)
                        ex_v = ex_t[:].rearrange("p (t h) -> p t h", h=HEADS)
                        rhs = wp.tile([P, Tmax * D1], fp16, tag="rhs")
                        rhs_v = rhs[:].rearrange("p (t d) -> p t d", d=D1)
                        nc.vector.tensor_copy(out=rhs_v[:, 0:T, C1:D1],
                                              in_=ex_v[:, 0:T, :])
                        for (k, co, n, so) in lofr:
                            nc.vector.tensor_tensor(
                                out=rhs_v[:, so:so + n, 0:C1].rearrange(
                                    "p t (c h) -> p t c h", h=HEADS),
                                in0=lo_tiles[k][:, co:co + n, 0:C1].rearrange(
                                    "p t (c h) -> p t c h", h=HEADS),
                                in1=ex_v[:, so:so + n, :].unsqueeze(2)
                                    .to_broadcast([P, n, HID, HEADS]),
                                op=AO.mult)
                        for (k, co, n, so) in hifr:
                            nc.vector.tensor_tensor(
                                out=rhs_v[:, T1 + so:T1 + so + n, 0:C1].rearrange(
                                    "p t (c h) -> p t c h", h=HEADS),
                                in0=hi_tiles[k][:, co:co + n, 0:C1].rearrange(
                                    "p t (c h) -> p t c h", h=HEADS),
                                in1=ex_v[:, T1 + so:T1 + so + n, :].unsqueeze(2)
                                    .to_broadcast([P, n, HID, HEADS]),
                                op=AO.mult)
                        ps1 = psB.tile([P, D1], fp32, tag="acc", space="PSUM")
                        for t in range(T):
                            eq = eqp.tile([P, P], fp16, tag="eq")
                            nc.vector.tensor_scalar(
                                out=eq[:], in0=iota_sb[:],
                                scalar1=dstl_sb[:, db + t:db + t + 1],
                                scalar2=None, op0=AO.is_equal)
                            nc.tensor.matmul(out=ps1[:], lhsT=eq[:],
                                             rhs=rhs[:, t * D1:(t + 1) * D1],
                                             start=(t == 0), stop=(t == T - 1))
                        nc.scalar.copy(
                            out=acc[:, (jj - jj0) * D1:(jj - jj0 + 1) * D1],
                            in_=ps1[:])
                    else:
                        ad_v = ad2c[:].rearrange("p (t d) -> p t d", d=2)
                        e_t = sm.tile([P, Tmax], fp16, tag="ec")
                        e_v = e_t[:].rearrange("p (t o) -> p t o", o=1)
                        for (k, co, n, so) in lofr:
                            nc.vector.tensor_tensor(
                                out=e_v[:, so:so + n, :],
                                in0=lo_tiles[k][:, co:co + n, N_CLS:N_CLS + 1],
                                in1=ad_v[:, db + so:db + so + n, 0:1], op=AO.add)
                        for (k, co, n, so) in hifr:
                            nc.vector.tensor_tensor(
                                out=e_v[:, T1 + so:T1 + so + n, :],
                                in0=hi_tiles[k][:, co:co + n, N_CLS:N_CLS + 1],
                                in1=ad_v[:, db + T1 + so:db + T1 + so + n, 0:1],
                                op=AO.add)
                        e2_t = sm.tile([P, Tmax], fp16, tag="e2c")
                        nc.vector.scalar_tensor_tensor(
                            out=e2_t[:, 0:T], in0=e_t[:, 0:T], scalar=NEG_SLOPE,
                            in1=e_t[:, 0:T], op0=AO.mult, op1=AO.max)
                        ex2_t = sm.tile([P, Tmax], fp32, tag="ex2")
                        nc.scalar.activation(out=ex2_t[:, 0:T], in_=e2_t[:, 0:T],
                                             func=AF.Exp)
                        ps2 = psB.tile([P, D2], fp32, tag="acc", space="PSUM")
                        t = 0
                        for (k, co, n, so) in lofr + hifr:
                            tl = lo_tiles if t < T1 else hi_tiles
                            for i in range(n):
                                eqw = eqp.tile([P, P], fp16, tag="eq")
                                nc.vector.tensor_scalar(
                                    out=eqw[:], in0=iota_sb[:],
                                    scalar1=dstl_sb[:, db + t:db + t + 1],
                                    scalar2=ex2_t[:, t:t + 1],
                                    op0=AO.is_equal, op1=AO.mult)
                                nc.tensor.matmul(
                                    out=ps2[:], lhsT=eqw[:],
                                    rhs=tl[k][:, co + i, :],
                                    start=(t == 0), stop=(t == T - 1))
                                t += 1
                        nc.scalar.copy(
                            out=acc[:, (jj - jj0) * D2:(jj - jj0 + 1) * D2],
                            in_=ps2[:])

                    if jj - jj0 + 1 == CB or jj == TPD - 1:
                        nb = jj - jj0 + 1
                        if conv == 1:
                            post1(acc, jj0, nb)
                            ck = jj // CB
                            for q in coll_after_chunk.get(ck, []):
                                emit_coll(q)
                        else:
                            post2(acc, jj0, nb)

            # ---------------- conv1 (collectives + a_d2 gathers interleaved)
            conv_pass(1)
            for q in range(NCOLL):
                emit_expand(q)
            # ---------------- conv2
            conv_pass(2)

    nc.compile()
    return nc


def _make_in_maps(inputs, plan, dev):
    x = np.asarray(inputs["x"], dtype=np.float32)
    W1 = np.asarray(inputs["W1"], dtype=np.float32)
    att_src1 = np.asarray(inputs["att_src1"], dtype=np.float32)
    att_dst1 = np.asarray(inputs["att_dst1"], dtype=np.float32)
    b1 = np.asarray(inputs["b1"], dtype=np.float32)
    W2 = np.asarray(inputs["W2"], dtype=np.float32)
    att_src2 = np.asarray(inputs["att_src2"], dtype=np.float32)
    att_dst2 = np.asarray(inputs["att_dst2"], dtype=np.float32)
    b2 = np.asarray(inputs["b2"], dtype=np.float32)

    As = np.zeros((C1, HEADS), np.float32)
    Ad = np.zeros((C1, HEADS), np.float32)
    for h in range(HEADS):
        As[h * HID:(h + 1) * HID, h] = att_src1[h]
        Ad[h * HID:(h + 1) * HID, h] = att_dst1[h]
    # c-major column permutation for the h block: col c*8+h <- h*16+c
    perm = np.arange(C1).reshape(HEADS, HID).T.reshape(-1)
    W1cm = W1[:, perm]
    w1ext = np.concatenate([W1cm, W1 @ As, W1 @ Ad], axis=1).astype(np.float16)
    w2ext = np.concatenate(
        [W2, (W2 @ att_src2[0])[:, None], (W2 @ att_dst2[0])[:, None]],
        axis=1).astype(np.float16)

    x_pad = np.zeros((N_PAD, F_IN), np.float32)
    x_pad[:N_NODES] = x
    xT = np.ascontiguousarray(x_pad.T.astype(np.float16))

    b1b = np.tile(b1[None, :], (P, 1)).astype(np.float32)
    b2b = np.tile(b2[None, :], (P, 1)).astype(np.float32)
    ident = np.eye(P, dtype=np.float16)
    iota = np.ascontiguousarray(
        np.tile(np.arange(P, dtype=np.float16)[None, :], (P, 1)))

    in_maps = []
    for d in range(N_DEV):
        src_widx, dst_widx, dstl2 = dev[d]
        in_maps.append({
            "xT": xT, "w1ext": w1ext, "w2ext": w2ext, "b1b": b1b, "b2b": b2b,
            "ident": ident, "iota": iota,
            "swidx": src_widx, "dwidx": dst_widx, "dstli": dstl2,
            "xT_loc": np.ascontiguousarray(xT[:, d * NPD:(d + 1) * NPD]),
        })
    return in_maps


def kernel(x, edge_index, W1, att_src1, att_dst1, b1, W2, att_src2, att_dst2, b2):
    edge_index = np.asarray(edge_index, dtype=np.int32)
    plan, dev = _plan(edge_index)

    key = (tuple(plan["T1u"]), tuple(plan["T2u"]))
    if key not in _CACHE:
        _CACHE[key] = _build(plan)
    nc = _CACHE[key]

    in_maps = _make_in_maps(dict(
        x=x, W1=W1, att_src1=att_src1, att_dst1=att_dst1, b1=b1,
        W2=W2, att_src2=att_src2, att_dst2=att_dst2, b2=b2), plan, dev)
    res = run_bass_kernel_spmd(nc, in_maps, list(range(N_DEV)))
    full = np.concatenate([res.results[d]["out"] for d in range(N_DEV)], axis=0)
    return full[:N_NODES]


# revision 19
# speedup vs baseline: 1.0466x; 1.0466x over previous
"""Trainium2 Bass kernel for a 2-layer GAT (PyG semantics, eval mode).

SPMD over 8 NeuronCores, dst-sorted edge partitioning:
 - conv aggregation + softmax denominators fused into one-hot matmuls
   (psum += eq.T @ rhs) per 128-edge tile.
 - Per-edge gathers via batched Q7 dma_gather (1024 descs/call, the ucode
   ring limit) with cost-optimal element sizes: conv1 h rows 512B, conv1
   a_d 16B, conv2 rows 86B, conv2 a_d2 4B.
 - Source-row gather calls are stream-packed (cross node-tile boundaries,
   per-slot reads fragmented); a_d gathers are slot-windowed (in-tile
   indices, narrow table deps) and prefetched into persistent buffers so
   they overlap phase 0 (conv1) / conv1 compute (conv2).
 - conv2 uses weighted one-hots (fused is_equal*exp) and a constant-1
   table column: no per-edge rhs assembly, denominator from the same
   matmul.
 - h = x @ W1ext computed fully redundantly per core (c-major head
   layout); h2 AllGather in 4 chunks interleaved with conv1 compute.
"""
import sys

sys.path.insert(0, "/opt/trn_rl_repo")

import numpy as np

import concourse.bacc as bacc
import concourse.bass as bass
import concourse.mybir as mybir
import concourse.tile as tile
from concourse.bass_utils import run_bass_kernel_spmd

P = 128
N_DEV = 8
N_NODES = 50000
F_IN, HID, N_CLS, HEADS = 128, 16, 40, 8
NEG_SLOPE = 0.2
HALF = 32768                 # int16 index limit for dma_gather

N_TILES = 392
N_PAD = N_TILES * P          # 50176
TPD = N_TILES // N_DEV       # 49
NPD = TPD * P                # 6272

C1 = HEADS * HID             # 128 (h block, c-major: col c*8+h)
D1 = C1 + HEADS              # 136 rhs cols conv1: [h*ex | ex]
R1 = 256                     # conv1 table row elems (fp16, 512B stride)
RL = 128                     # conv2 table / ad_win row elems (256B stride)
D2 = N_CLS + 3               # 43 conv2 row: [h2lin(40)|a_s2|a_d2|one]
CMP = 44                     # compact collective row (43 used)
W2C = N_CLS + 2              # 42 = [W2 | w_as2 | w_ad2]

CT = 8                       # tiles per gather call (1024 descs: ring limit)
CB = 7                       # node tiles per post-processing chunk
NCOLL = 4                    # collective chunks
SLC = 8                      # src-idx calls per sidx load chunk

fp32 = mybir.dt.float32
fp16 = mybir.dt.float16
i16 = mybir.dt.int16

_CACHE = {}


def _patched_dma_gather():
    import inspect, textwrap
    src = inspect.getsource(bass.BassGpSimd.dma_gather)
    old = """        assert (
            elem_size_bytes > 0 and elem_size_bytes % 256 == 0
        )  # transpose restriction"""
    assert old in src, "bass dma_gather source changed; fall back to 256B elems"
    src = src.replace(old, """        assert elem_size_bytes > 0
        if transpose:
            assert elem_size_bytes % 256 == 0""")
    g = dict(bass.__dict__)
    exec(compile(textwrap.dedent(src), "<patched_dma_gather>", "exec"), g)
    return g["dma_gather"]


_PG = _patched_dma_gather()


def _wrap(idx_list):
    """int16 idx list -> [128, n/16] wrapped layout (j at [j%16, j//16],
    replicated across the 8 16-partition groups)."""
    n = len(idx_list)
    assert n % 16 == 0
    w = np.asarray(idx_list, dtype=np.int16).reshape(n // 16, 16).T
    return np.tile(w, (8, 1))


def _frags(cum, jj):
    """Split stream tiles [cum[jj], cum[jj+1]) at CT boundaries.
    Returns [(call, off_in_call, n, off_in_slot)]."""
    s0, s1 = int(cum[jj]), int(cum[jj + 1])
    res = []
    s = s0
    while s < s1:
        k = s // CT
        n = min((k + 1) * CT, s1) - s
        res.append((k, s - k * CT, n, s - s0))
        s += n
    return res


def _plan(edge_index):
    """Shared (cross-device) program plan + per-device data arrays."""
    src = np.concatenate([edge_index[0], np.arange(N_NODES, dtype=np.int32)])
    dst = np.concatenate([edge_index[1], np.arange(N_NODES, dtype=np.int32)])
    order = np.argsort(dst, kind="stable")
    src_s, dst_s = src[order], dst[order]
    bounds = np.searchsorted(dst_s, np.arange(N_TILES + 1) * P).astype(np.int64)

    lo_lists, hi_lists = [], []
    for j in range(N_TILES):
        s = src_s[bounds[j]:bounds[j + 1]]
        d = dst_s[bounds[j]:bounds[j + 1]]
        m = s < HALF
        lo_lists.append((s[m], d[m]))
        hi_lists.append((s[~m] - HALF, d[~m]))

    t1 = np.array([(len(lo_lists[j][0]) + P - 1) // P for j in range(N_TILES)])
    t2 = np.array([(len(hi_lists[j][0]) + P - 1) // P for j in range(N_TILES)])
    T1u = t1.reshape(N_DEV, TPD).max(axis=0)           # per-slot max
    T2u = t2.reshape(N_DEV, TPD).max(axis=0)
    Tu = T1u + T2u
    n_et = int(Tu.sum())

    lo_cum = np.concatenate([[0], np.cumsum(T1u)]).astype(int)
    hi_cum = np.concatenate([[0], np.cumsum(T2u)]).astype(int)
    d_cum = np.concatenate([[0], np.cumsum(Tu)]).astype(int)
    LOT, HIT = int(lo_cum[-1]), int(hi_cum[-1])
    n_lo_calls = (LOT + CT - 1) // CT
    n_hi_calls = (HIT + CT - 1) // CT

    plan = dict(T1u=T1u, T2u=T2u, Tu=Tu, n_et=n_et,
                lo_cum=lo_cum, hi_cum=hi_cum, d_cum=d_cum,
                LOT=LOT, HIT=HIT, n_lo_calls=n_lo_calls, n_hi_calls=n_hi_calls)

    dev = []
    for d in range(N_DEV):
        lo_stream = np.zeros(n_lo_calls * CT * P, np.int32)
        hi_stream = np.zeros(n_hi_calls * CT * P, np.int32)
        dit = np.zeros(n_et * P, np.int32)      # in-tile dst idx (0..127)
        dstl = np.full(n_et * P, 300.0, np.float32)
        for jj in range(TPD):
            j = d * TPD + jj
            sl, dl = lo_lists[j]
            sh, dh = hi_lists[j]
            lo_stream[lo_cum[jj] * P:lo_cum[jj] * P + len(sl)] = sl
            hi_stream[hi_cum[jj] * P:hi_cum[jj] * P + len(sh)] = sh
            base = d_cum[jj] * P
            dit[base:base + len(dl)] = dl - j * P
            dit[base + T1u[jj] * P:base + T1u[jj] * P + len(dh)] = dh - j * P
            dstl[base:base + len(dl)] = dl - j * P
            dstl[base + T1u[jj] * P:base + T1u[jj] * P + len(dh)] = dh - j * P
        sblocks = []
        for k in range(n_lo_calls):
            sblocks.append(_wrap(lo_stream[k * CT * P:(k + 1) * CT * P]))
        for k in range(n_hi_calls):
            sblocks.append(_wrap(hi_stream[k * CT * P:(k + 1) * CT * P]))
        src_widx = np.ascontiguousarray(
            np.concatenate(sblocks, axis=1)).astype(np.int16)
        dblocks = []
        for jj in range(TPD):
            T = int(Tu[jj])
            base = d_cum[jj] * P
            for c0 in range(0, T, CT):
                n = min(CT, T - c0)
                dblocks.append(_wrap(dit[base + c0 * P:base + (c0 + n) * P]))
        dst_widx = np.ascontiguousarray(
            np.concatenate(dblocks, axis=1)).astype(np.int16)
        dstl2 = np.ascontiguousarray(dstl.reshape(n_et, P).T)
        dev.append((src_widx, dst_widx, dstl2))
    return plan, dev


def _build(plan):
    T1u, T2u, Tu = plan["T1u"], plan["T2u"], plan["Tu"]
    n_et = plan["n_et"]
    lo_cum, hi_cum, d_cum = plan["lo_cum"], plan["hi_cum"], plan["d_cum"]
    LOT, HIT = plan["LOT"], plan["HIT"]
    n_lo_calls, n_hi_calls = plan["n_lo_calls"], plan["n_hi_calls"]
    Tmax = int(Tu.max())

    lo_col = lambda k: k * CT * 8
    hi_col = lambda k: (n_lo_calls + k) * CT * 8
    SRC_COLS = (n_lo_calls + n_hi_calls) * CT * 8
    dw_col = {}
    c = 0
    for jj in range(TPD):
        T = int(Tu[jj])
        for c0 in range(0, T, CT):
            n = min(CT, T - c0)
            dw_col[(jj, c0)] = c
            c += n * 8
    DST_COLS = c

    cbnd = [0, 14, 28, 42, TPD]
    coll_after_chunk = {2: [0], 4: [1], 5: [2], 6: [3]}

    nc = bacc.Bacc("TRN2", target_bir_lowering=False, debug=False,
                   num_devices=N_DEV, num_swdge_queues=4)

    xT = nc.dram_tensor("xT", [P, N_PAD], fp16, kind="ExternalInput")
    xT_loc = nc.dram_tensor("xT_loc", [P, NPD], fp16, kind="ExternalInput")
    w1ext = nc.dram_tensor("w1ext", [P, D1 + HEADS], fp16, kind="ExternalInput")
    w2ext = nc.dram_tensor("w2ext", [HID, W2C], fp16, kind="ExternalInput")
    b1b = nc.dram_tensor("b1b", [P, HID], fp32, kind="ExternalInput")
    b2b = nc.dram_tensor("b2b", [P, N_CLS], fp32, kind="ExternalInput")
    ident = nc.dram_tensor("ident", [P, P], fp16, kind="ExternalInput")
    iota = nc.dram_tensor("iota", [P, P], fp16, kind="ExternalInput")
    swidx = nc.dram_tensor("swidx", [P, SRC_COLS], i16, kind="ExternalInput")
    dwidx = nc.dram_tensor("dwidx", [P, DST_COLS], i16, kind="ExternalInput")
    dstli = nc.dram_tensor("dstli", [P, n_et], fp32, kind="ExternalInput")
    out = nc.dram_tensor("out", [NPD, N_CLS], fp32, kind="ExternalOutput")

    h_lo_t = nc.dram_tensor("h_lo_t", [HALF, R1], fp16)
    h_hi_t = nc.dram_tensor("h_hi_t", [N_PAD - HALF, R1], fp16)
    ad_win = nc.dram_tensor("ad_win", [NPD, RL], fp16)
    h2_loc = nc.dram_tensor("h2_loc", [NPD, RL], fp16)
    h2_cmp_loc = nc.dram_tensor("h2_cmp_loc", [NPD, CMP], fp16)
    h2_gath = nc.dram_tensor("h2_gath", [N_PAD, CMP], fp16, addr_space="Shared")
    h2_ext = nc.dram_tensor("h2_ext", [N_PAD, RL], fp16)

    rg = [list(range(N_DEV))]
    AO = mybir.AluOpType
    AF = mybir.ActivationFunctionType
    AX = mybir.AxisListType.X
    _q = [0]

    def qn():
        _q[0] = (_q[0] + 1) % 4
        return _q[0]

    with tile.TileContext(nc) as tc:
        with (
            tc.tile_pool(name="const", bufs=1) as cpool,
            tc.tile_pool(name="glo", bufs=8) as glo,
            tc.tile_pool(name="ghi", bufs=7) as ghi,
            tc.tile_pool(name="sidx", bufs=2) as sxp,
            tc.tile_pool(name="work", bufs=3) as wp,
            tc.tile_pool(name="eqp", bufs=32) as eqp,
            tc.tile_pool(name="small", bufs=2) as sm,
            tc.tile_pool(name="psA", bufs=3, space="PSUM") as psA,
            tc.tile_pool(name="psB", bufs=3, space="PSUM") as psB,
            tc.tile_pool(name="psC", bufs=2, space="PSUM") as psC,
        ):
            # ---------------- constants
            w1_sb = cpool.tile([P, D1 + HEADS], fp16)
            nc.sync.dma_start(out=w1_sb[:], in_=w1ext[:])
            w2_sb = cpool.tile([HID, W2C], fp16)
            nc.sync.dma_start(out=w2_sb[:], in_=w2ext[:])
            b1_sb = cpool.tile([P, HID], fp32)
            nc.sync.dma_start(out=b1_sb[:], in_=b1b[:])
            b2_sb = cpool.tile([P, N_CLS], fp32)
            nc.sync.dma_start(out=b2_sb[:], in_=b2b[:])
            id_sb = cpool.tile([P, P], fp16)
            nc.sync.dma_start(out=id_sb[:], in_=ident[:])
            iota_sb = cpool.tile([P, P], fp16)
            nc.sync.dma_start(out=iota_sb[:], in_=iota[:])
            dwidx_sb = cpool.tile([P, DST_COLS], i16)
            nc.sync.dma_start(out=dwidx_sb[:], in_=dwidx[:])
            dstl_sb = cpool.tile([P, n_et], fp32)
            nc.sync.dma_start(out=dstl_sb[:], in_=dstli[:])
            ones_sb = cpool.tile([P, 1], fp16)
            nc.vector.memset(ones_sb[:], 1.0)
            h1all = cpool.tile([P, TPD * HID], fp16)
            ad1c = cpool.tile([P, n_et * HEADS], fp16)   # conv1 a_d per edge
            ad2c = cpool.tile([P, n_et * 2], fp16)       # conv2 a_d2 per edge

            # ---------------- ad_win: local a_d rows from xT_loc
            hst2 = cpool.tile([P, TPD * HEADS], fp16)
            NC2 = 7
            for cc in range(TPD // NC2):
                xc2 = wp.tile([P, NC2 * P], fp16, tag="xc")
                nc.sync.dma_start(
                    out=xc2[:], in_=xT_loc[:, cc * NC2 * P:(cc + 1) * NC2 * P])
                psd = psB.tile([P, NC2 * HEADS], fp32, tag="acc", space="PSUM")
                for k in range(NC2):
                    nc.tensor.matmul(out=psd[:, k * HEADS:(k + 1) * HEADS],
                                     lhsT=xc2[:, k * P:(k + 1) * P],
                                     rhs=w1_sb[:, D1:D1 + HEADS],
                                     start=True, stop=True)
                nc.scalar.copy(out=hst2[:, cc * NC2 * HEADS:(cc + 1) * NC2 * HEADS],
                               in_=psd[:])
            nc.sync.dma_start(
                out=ad_win[:, 0:HEADS].rearrange("(t p) d -> p t d", p=P),
                in_=hst2[:].rearrange("p (t d) -> p t d", d=HEADS))

            # conv1 a_d prefetch calls: emitted interleaved with phase 0
            ad1_calls = [(jj, c0) for jj in range(TPD)
                         for c0 in range(0, int(Tu[jj]), CT)]
            ad1_pos = [0]

            def emit_ad1(k):
                for (jj, c0) in ad1_calls[ad1_pos[0]:ad1_pos[0] + k]:
                    n = min(CT, int(Tu[jj]) - c0)
                    dc = dw_col[(jj, c0)]
                    _PG(nc.gpsimd,
                        out_ap=ad1c[:].rearrange("p (t d) -> p t d", d=HEADS)[
                            :, int(d_cum[jj]) + c0:int(d_cum[jj]) + c0 + n, :],
                        in_ap=ad_win[jj * P:(jj + 1) * P, 0:HEADS],
                        idxs_ap=dwidx_sb[:, dc:dc + n * 8],
                        num_idxs=n * P, num_idxs_reg=n * P,
                        elem_size=HEADS, elem_step=RL, queue_num=qn())
                ad1_pos[0] += k

            # ---------------- phase 0: full-redundant h table (c-major rows)
            NCHUNK = 8
            for cc in range(N_TILES // NCHUNK):
                xc = wp.tile([P, NCHUNK * P], fp16, tag="xc")
                nc.sync.dma_start(
                    out=xc[:], in_=xT[:, cc * NCHUNK * P:(cc + 1) * NCHUNK * P])
                hst = wp.tile([P, NCHUNK * R1], fp16, tag="hst")
                for gi, (g0, gn) in enumerate(((0, 3), (3, 3), (6, 2))):
                    psh = psA.tile([P, 3 * (D1 + HEADS)], fp32, tag="big",
                                   space="PSUM")
                    for k in range(gn):
                        nc.tensor.matmul(
                            out=psh[:, k * (D1 + HEADS):(k + 1) * (D1 + HEADS)],
                            lhsT=xc[:, (g0 + k) * P:(g0 + k + 1) * P],
                            rhs=w1_sb[:], start=True, stop=True)
                    dst_view = hst[:].rearrange("p (k d) -> p k d", d=R1)[
                        :, g0:g0 + gn, 0:D1 + HEADS]
                    src_view = psh[:].rearrange("p (k d) -> p k d",
                                                d=D1 + HEADS)[:, 0:gn, :]
                    if gi % 2 == 0:
                        nc.scalar.copy(out=dst_view, in_=src_view)
                    else:
                        nc.vector.tensor_copy(out=dst_view, in_=src_view)
                r0 = cc * NCHUNK * P
                tgt = (h_lo_t[r0:r0 + NCHUNK * P, :] if r0 < HALF
                       else h_hi_t[r0 - HALF:r0 - HALF + NCHUNK * P, :])
                nc.sync.dma_start(
                    out=tgt.rearrange("(k p) d -> p k d", p=P),
                    in_=hst[:].rearrange("p (k d) -> p k d", d=R1))
                emit_ad1(4)

            emit_ad1(len(ad1_calls) - ad1_pos[0])

            # ---- conv1 post: ELU(mean(agg/den) + b1) -> h1all, then h2 rows
            def post1(acc, jj0, nb):
                a_v = acc[:].rearrange("p (b d) -> p b d", d=D1)
                den = sm.tile([P, CB * HEADS], fp32, tag="den")
                nc.vector.tensor_scalar(
                    out=den[:].rearrange("p (b h) -> p b h", h=HEADS)[:, 0:nb, :],
                    in0=a_v[:, 0:nb, C1:D1], scalar1=1e-16, scalar2=None,
                    op0=AO.add)
                rec = sm.tile([P, CB * HEADS], fp32, tag="rec")
                nc.vector.reciprocal(out=rec[:, 0:nb * HEADS],
                                     in_=den[:, 0:nb * HEADS])
                nc.vector.tensor_scalar(out=rec[:, 0:nb * HEADS],
                                        in0=rec[:, 0:nb * HEADS],
                                        scalar1=1.0 / HEADS, scalar2=None,
                                        op0=AO.mult)
                tmp = sm.tile([P, CB * C1], fp32, tag="tmp")
                nc.vector.tensor_tensor(
                    out=tmp[:].rearrange("p (b c h) -> p b c h",
                                         c=HID, h=HEADS)[:, 0:nb],
                    in0=a_v[:, 0:nb, 0:C1].rearrange("p b (c h) -> p b c h",
                                                     h=HEADS),
                    in1=rec[:].rearrange("p (b h) -> p b h", h=HEADS)[:, 0:nb, :]
                        .unsqueeze(2).to_broadcast([P, nb, HID, HEADS]),
                    op=AO.mult)
                h1b = sm.tile([P, CB * HID], fp32, tag="h1b")
                nc.vector.tensor_reduce(
                    out=h1b[:].rearrange("p (b c) -> p b c", c=HID)[:, 0:nb, :],
                    in_=tmp[:].rearrange("p (b c h) -> p b c h",
                                         c=HID, h=HEADS)[:, 0:nb],
                    axis=AX, op=AO.add)
                nc.vector.tensor_tensor(
                    out=h1b[:].rearrange("p (b c) -> p b c", c=HID)[:, 0:nb, :],
                    in0=h1b[:].rearrange("p (b c) -> p b c", c=HID)[:, 0:nb, :],
                    in1=b1_sb[:].unsqueeze(1).to_broadcast([P, nb, HID]),
                    op=AO.add)
                xm = sm.tile([P, CB * HID], fp32, tag="xm")
                nc.vector.tensor_scalar(out=xm[:, 0:nb * HID],
                                        in0=h1b[:, 0:nb * HID],
                                        scalar1=0.0, scalar2=None, op0=AO.min)
                em = sm.tile([P, CB * HID], fp32, tag="em")
                nc.scalar.activation(out=em[:, 0:nb * HID], in_=xm[:, 0:nb * HID],
                                     func=AF.Exp)
                xp = sm.tile([P, CB * HID], fp32, tag="xp")
                nc.vector.tensor_scalar(out=xp[:, 0:nb * HID],
                                        in0=h1b[:, 0:nb * HID],
                                        scalar1=0.0, scalar2=None, op0=AO.max)
                h1f = sm.tile([P, CB * HID], fp32, tag="h1f")
                nc.vector.tensor_tensor(out=h1f[:, 0:nb * HID],
                                        in0=em[:, 0:nb * HID],
                                        in1=xp[:, 0:nb * HID], op=AO.add)
                nc.vector.tensor_scalar(out=h1all[:, jj0 * HID:(jj0 + nb) * HID],
                                        in0=h1f[:, 0:nb * HID],
                                        scalar1=-1.0, scalar2=None, op0=AO.add)
                for i in range(nb):
                    jj = jj0 + i
                    pst = psC.tile([HID, P], fp16, tag="tp", space="PSUM")
                    nc.tensor.transpose(out=pst[:],
                                        in_=h1all[:, jj * HID:(jj + 1) * HID],
                                        identity=id_sb[:])
                    h1T = sm.tile([HID, P], fp16, tag="h1T")
                    nc.scalar.copy(out=h1T[:], in_=pst[:])
                    psh2 = psC.tile([P, W2C], fp32, tag="tp", space="PSUM")
                    nc.tensor.matmul(out=psh2[:], lhsT=h1T[:], rhs=w2_sb[:],
                                     start=True, stop=True)
                    h2st = sm.tile([P, CMP], fp16, tag="h2st")
                    nc.scalar.copy(out=h2st[:, 0:W2C], in_=psh2[:])
                    nc.vector.tensor_copy(out=h2st[:, W2C:W2C + 1], in_=ones_sb[:])
                    nc.sync.dma_start(out=h2_loc[jj * P:(jj + 1) * P, 0:CMP],
                                      in_=h2st[:])
                    nc.sync.dma_start(out=h2_cmp_loc[jj * P:(jj + 1) * P, :],
                                      in_=h2st[:])
                # conv2 a_d2 gathers for these tiles (overlap conv1/collective)
                for i in range(nb):
                    jj = jj0 + i
                    T = int(Tu[jj])
                    for c0 in range(0, T, CT):
                        n = min(CT, T - c0)
                        dc = dw_col[(jj, c0)]
                        _PG(nc.gpsimd,
                            out_ap=ad2c[:].rearrange("p (t d) -> p t d", d=2)[
                                :, int(d_cum[jj]) + c0:int(d_cum[jj]) + c0 + n, :],
                            in_ap=h2_loc[jj * P:(jj + 1) * P,
                                         N_CLS + 1:N_CLS + 3],
                            idxs_ap=dwidx_sb[:, dc:dc + n * 8],
                            num_idxs=n * P, num_idxs_reg=n * P,
                            elem_size=2, elem_step=RL, queue_num=qn())

            # ---- conv2 post: log_softmax(agg/den + b2) -> out
            def post2(acc, jj0, nb):
                a_v = acc[:].rearrange("p (b d) -> p b d", d=D2)
                den = sm.tile([P, CB], fp32, tag="den2")
                nc.vector.tensor_scalar(
                    out=den[:].rearrange("p (b o) -> p b o", o=1)[:, 0:nb, :],
                    in0=a_v[:, 0:nb, D2 - 1:D2], scalar1=1e-16, scalar2=None,
                    op0=AO.add)
                rec = sm.tile([P, CB], fp32, tag="rec2")
                nc.vector.reciprocal(out=rec[:, 0:nb], in_=den[:, 0:nb])
                h2f = sm.tile([P, CB * N_CLS], fp32, tag="h2f")
                h2f_v = h2f[:].rearrange("p (b c) -> p b c", c=N_CLS)
                nc.vector.tensor_tensor(
                    out=h2f_v[:, 0:nb, :], in0=a_v[:, 0:nb, 0:N_CLS],
                    in1=rec[:].rearrange("p (b o) -> p b o", o=1)[:, 0:nb, :]
                        .to_broadcast([P, nb, N_CLS]),
                    op=AO.mult)
                nc.vector.tensor_tensor(
                    out=h2f_v[:, 0:nb, :], in0=h2f_v[:, 0:nb, :],
                    in1=b2_sb[:].unsqueeze(1).to_broadcast([P, nb, N_CLS]),
                    op=AO.add)
                nm = sm.tile([P, CB], fp32, tag="nm")
                nc.vector.tensor_reduce(
                    out=nm[:].rearrange("p (b o) -> p b o", o=1)[:, 0:nb, :],
                    in_=h2f_v[:, 0:nb, :], axis=AX, op=AO.max, negate=True)
                hs = sm.tile([P, CB * N_CLS], fp32, tag="hs")
                hs_v = hs[:].rearrange("p (b c) -> p b c", c=N_CLS)
                nc.vector.tensor_tensor(
                    out=hs_v[:, 0:nb, :], in0=h2f_v[:, 0:nb, :],
                    in1=nm[:].rearrange("p (b o) -> p b o", o=1)[:, 0:nb, :]
                        .to_broadcast([P, nb, N_CLS]),
                    op=AO.add)
                es = sm.tile([P, CB * N_CLS], fp32, tag="es")
                nc.scalar.activation(out=es[:, 0:nb * N_CLS],
                                     in_=hs[:, 0:nb * N_CLS], func=AF.Exp)
                ssum = sm.tile([P, CB], fp32, tag="ssum")
                nc.vector.tensor_reduce(
                    out=ssum[:].rearrange("p (b o) -> p b o", o=1)[:, 0:nb, :],
                    in_=es[:].rearrange("p (b c) -> p b c", c=N_CLS)[:, 0:nb, :],
                    axis=AX, op=AO.add)
                lg = sm.tile([P, CB], fp32, tag="lg")
                nc.scalar.activation(out=lg[:, 0:nb], in_=ssum[:, 0:nb],
                                     func=AF.Ln)
                ot = sm.tile([P, CB * N_CLS], fp32, tag="ot")
                nc.vector.tensor_tensor(
                    out=ot[:].rearrange("p (b c) -> p b c", c=N_CLS)[:, 0:nb, :],
                    in0=hs_v[:, 0:nb, :],
                    in1=lg[:].rearrange("p (b o) -> p b o", o=1)[:, 0:nb, :]
                        .to_broadcast([P, nb, N_CLS]),
                    op=AO.subtract)
                nc.sync.dma_start(
                    out=out[jj0 * P:(jj0 + nb) * P, :]
                        .rearrange("(b p) d -> p b d", p=P),
                    in_=ot[:].rearrange("p (b c) -> p b c", c=N_CLS)[:, 0:nb, :])

            def emit_coll(q):
                r0, r1 = cbnd[q] * P, cbnd[q + 1] * P
                nc.gpsimd.collective_compute(
                    "AllGather", AO.bypass, replica_groups=rg,
                    ins=[h2_cmp_loc[r0:r1, :].opt()],
                    outs=[h2_gath[r0 * N_DEV:r1 * N_DEV, :].opt()])

            def emit_expand(q):
                r0, r1 = cbnd[q] * P, cbnd[q + 1] * P
                nc.sync.dma_start(
                    out=h2_ext[:, 0:CMP]
                        .rearrange("(d r) c -> d r c", d=N_DEV)[:, r0:r1, :],
                    in_=h2_gath[r0 * N_DEV:r1 * N_DEV, :]
                        .rearrange("(d r) c -> d r c", d=N_DEV, r=r1 - r0))

            # ---------------- shared conv loop
            def conv_pass(conv):
                RW = R1 if conv == 1 else D2
                lo_tiles = {}
                hi_tiles = {}
                next_lo = next_hi = 0
                state = {"lo": [None, 0, -1], "hi": [None, 0, -1]}  # tile,c0,c1

                def load_sidx(st, col0):
                    col1 = min(col0 + SLC * CT * 8, SRC_COLS)
                    t = sxp.tile([P, SLC * CT * 8], i16, tag="sidx_" + st)
                    nc.sync.dma_start(out=t[:, 0:col1 - col0],
                                      in_=swidx[:, col0:col1])
                    state[st] = [t, col0, col1]

                def idx_view(st, c0, ncols):
                    t, s0, s1 = state[st]
                    if t is None or c0 < s0 or c0 + ncols > s1:
                        load_sidx(st, c0)
                        t, s0, s1 = state[st]
                    return t[:, c0 - s0:c0 - s0 + ncols]

                def emit_lo(k):
                    nonlocal next_lo
                    nt = min(CT, LOT - k * CT)
                    iv = idx_view("lo", lo_col(k), nt * 8)
                    t = glo.tile([P, CT * RW], fp16,
                                 tag="rlo" if conv == 1 else "rlo2")
                    v = t[:].rearrange("p (t d) -> p t d", d=RW)
                    if conv == 1:
                        _PG(nc.gpsimd, out_ap=v[:, 0:nt, :], in_ap=h_lo_t[:, :],
                            idxs_ap=iv, num_idxs=nt * P, num_idxs_reg=nt * P,
                            elem_size=R1, queue_num=qn())
                    else:
                        _PG(nc.gpsimd, out_ap=v[:, 0:nt, :],
                            in_ap=h2_ext[0:HALF, 0:D2],
                            idxs_ap=iv, num_idxs=nt * P, num_idxs_reg=nt * P,
                            elem_size=D2, elem_step=RL, queue_num=qn())
                    lo_tiles[k] = v
                    next_lo = k + 1

                def emit_hi(k):
                    nonlocal next_hi
                    nt = min(CT, HIT - k * CT)
                    iv = idx_view("hi", hi_col(k), nt * 8)
                    t = ghi.tile([P, CT * RW], fp16,
                                 tag="rhi" if conv == 1 else "rhi2")
                    v = t[:].rearrange("p (t d) -> p t d", d=RW)
                    if conv == 1:
                        _PG(nc.gpsimd, out_ap=v[:, 0:nt, :], in_ap=h_hi_t[:, :],
                            idxs_ap=iv, num_idxs=nt * P, num_idxs_reg=nt * P,
                            elem_size=R1, queue_num=qn())
                    else:
                        _PG(nc.gpsimd, out_ap=v[:, 0:nt, :],
                            in_ap=h2_ext[HALF:N_PAD, 0:D2],
                            idxs_ap=iv, num_idxs=nt * P, num_idxs_reg=nt * P,
                            elem_size=D2, elem_step=RL, queue_num=qn())
                    hi_tiles[k] = v
                    next_hi = k + 1

                acc = None
                jj0 = 0
                for jj in range(TPD):
                    T1, T2, T = int(T1u[jj]), int(T2u[jj]), int(Tu[jj])
                    lofr = _frags(lo_cum, jj)
                    hifr = _frags(hi_cum, jj)
                    while next_lo * CT < lo_cum[jj + 1]:
                        emit_lo(next_lo)
                    while next_hi * CT < hi_cum[jj + 1]:
                        emit_hi(next_hi)

                    if jj % CB == 0:
                        jj0 = jj
                        acc = sm.tile([P, CB * (D1 if conv == 1 else D2)], fp32,
                                      tag="acc1" if conv == 1 else "acc2")

                    db = int(d_cum[jj])
                    if conv == 1:
                        ad_v = ad1c[:].rearrange("p (t d) -> p t d", d=HEADS)
                        e_t = sm.tile([P, Tmax * HEADS], fp16, tag="e")
                        e_v = e_t[:].rearrange("p (t h) -> p t h", h=HEADS)
                        for (k, co, n, so) in lofr:
                            nc.vector.tensor_tensor(
                                out=e_v[:, so:so + n, :],
                                in0=lo_tiles[k][:, co:co + n, C1:C1 + HEADS],
                                in1=ad_v[:, db + so:db + so + n, :], op=AO.add)
                        for (k, co, n, so) in hifr:
                            nc.vector.tensor_tensor(
                                out=e_v[:, T1 + so:T1 + so + n, :],
                                in0=hi_tiles[k][:, co:co + n, C1:C1 + HEADS],
                                in1=ad_v[:, db + T1 + so:db + T1 + so + n, :],
                                op=AO.add)
                        e2_t = sm.tile([P, Tmax * HEADS], fp16, tag="e2")
                        nc.vector.scalar_tensor_tensor(
                            out=e2_t[:, 0:T * HEADS], in0=e_t[:, 0:T * HEADS],
                            scalar=NEG_SLOPE, in1=e_t[:, 0:T * HEADS],
                            op0=AO.mult, op1=AO.max)
                        ex_t = sm.tile([P, Tmax * HEADS], fp16, tag="ex")
                        nc.scalar.activation(out=ex_t[:, 0:T * HEADS],
                                             in_=e2_t[:, 0:T * HEADS],
                                             func=AF.Exp)
                        ex_v = ex_t[:].rearrange("p (t h) -> p t h", h=HEADS)
                        rhs = wp.tile([P, Tmax * D1], fp16, tag="rhs")
                        rhs_v = rhs[:].rearrange("p (t d) -> p t d", d=D1)
                        nc.vector.tensor_copy(out=rhs_v[:, 0:T, C1:D1],
                                              in_=ex_v[:, 0:T, :])
                        for (k, co, n, so) in lofr:
                            nc.vector.tensor_tensor(
                                out=rhs_v[:, so:so + n, 0:C1].rearrange(
                                    "p t (c h) -> p t c h", h=HEADS),
                                in0=lo_tiles[k][:, co:co + n, 0:C1].rearrange(
                                    "p t (c h) -> p t c h", h=HEADS),
                                in1=ex_v[:, so:so + n, :].unsqueeze(2)
                                    .to_broadcast([P, n, HID, HEADS]),
                                op=AO.mult)
                        for (k, co, n, so) in hifr:
                            nc.vector.tensor_tensor(
                                out=rhs_v[:, T1 + so:T1 + so + n, 0:C1].rearrange(
                                    "p t (c h) -> p t c h", h=HEADS),
                                in0=hi_tiles[k][:, co:co + n, 0:C1].rearrange(
                                    "p t (c h) -> p t c h", h=HEADS),
                                in1=ex_v[:, T1 + so:T1 + so + n, :].unsqueeze(2)
                                    .to_broadcast([P, n, HID, HEADS]),
                                op=AO.mult)
                        ps1 = psB.tile([P, D1], fp32, tag="acc", space="PSUM")
                        for t in range(T):
                            eq = eqp.tile([P, P], fp16, tag="eq")
                            nc.vector.tensor_scalar(
                                out=eq[:], in0=iota_sb[:],
                                scalar1=dstl_sb[:, db + t:db + t + 1],
                                scalar2=None, op0=AO.is_equal)
                            nc.tensor.matmul(out=ps1[:], lhsT=eq[:],
                                             rhs=rhs[:, t * D1:(t + 1) * D1],
                                             start=(t == 0), stop=(t == T - 1))
                        nc.scalar.copy(
                            out=acc[:, (jj - jj0) * D1:(jj - jj0 + 1) * D1],
                            in_=ps1[:])
                    else:
                        ad_v = ad2c[:].rearrange("p (t d) -> p t d", d=2)
                        e_t = sm.tile([P, Tmax], fp16, tag="ec")
                        e_v = e_t[:].rearrange("p (t o) -> p t o", o=1)
                        for (k, co, n, so) in lofr:
                            nc.vector.tensor_tensor(
                                out=e_v[:, so:so + n, :],
                                in0=lo_tiles[k][:, co:co + n, N_CLS:N_CLS + 1],
                                in1=ad_v[:, db + so:db + so + n, 0:1], op=AO.add)
                        for (k, co, n, so) in hifr:
                            nc.vector.tensor_tensor(
                                out=e_v[:, T1 + so:T1 + so + n, :],
                                in0=hi_tiles[k][:, co:co + n, N_CLS:N_CLS + 1],
                                in1=ad_v[:, db + T1 + so:db + T1 + so + n, 0:1],
                                op=AO.add)
                        e2_t = sm.tile([P, Tmax], fp16, tag="e2c")
                        nc.vector.scalar_tensor_tensor(
                            out=e2_t[:, 0:T], in0=e_t[:, 0:T], scalar=NEG_SLOPE,
                            in1=e_t[:, 0:T], op0=AO.mult, op1=AO.max)
                        ex2_t = sm.tile([P, Tmax], fp32, tag="ex2")
                        nc.scalar.activation(out=ex2_t[:, 0:T], in_=e2_t[:, 0:T],
                                             func=AF.Exp)
                        ps2 = psB.tile([P, D2], fp32, tag="acc", space="PSUM")
                        t = 0
                        for (k, co, n, so) in lofr + hifr:
                            tl = lo_tiles if t < T1 else hi_tiles
                            for i in range(n):
                                eqw = eqp.tile([P, P], fp16, tag="eq")
                                nc.vector.tensor_scalar(
                                    out=eqw[:], in0=iota_sb[:],
                                    scalar1=dstl_sb[:, db + t:db + t + 1],
                                    scalar2=ex2_t[:, t:t + 1],
                                    op0=AO.is_equal, op1=AO.mult)
                                nc.tensor.matmul(
                                    out=ps2[:], lhsT=eqw[:],
                                    rhs=tl[k][:, co + i, :],
                                    start=(t == 0), stop=(t == T - 1))
                                t += 1
                        nc.scalar.copy(
                            out=acc[:, (jj - jj0) * D2:(jj - jj0 + 1) * D2],
                            in_=ps2[:])

                    if jj - jj0 + 1 == CB or jj == TPD - 1:
                        nb = jj - jj0 + 1
                        if conv == 1:
                            post1(acc, jj0, nb)
                            ck = jj // CB
                            for q in coll_after_chunk.get(ck, []):
                                emit_coll(q)
                        else:
                            post2(acc, jj0, nb)

            # ---------------- conv1 (collectives + a_d2 gathers interleaved)
            conv_pass(1)
            for q in range(NCOLL):
                emit_expand(q)
            # ---------------- conv2
            conv_pass(2)

    nc.compile()
    return nc


def _make_in_maps(inputs, plan, dev):
    x = np.asarray(inputs["x"], dtype=np.float32)
    W1 = np.asarray(inputs["W1"], dtype=np.float32)
    att_src1 = np.asarray(inputs["att_src1"], dtype=np.float32)
    att_dst1 = np.asarray(inputs["att_dst1"], dtype=np.float32)
    b1 = np.asarray(inputs["b1"], dtype=np.float32)
    W2 = np.asarray(inputs["W2"], dtype=np.float32)
    att_src2 = np.asarray(inputs["att_src2"], dtype=np.float32)
    att_dst2 = np.asarray(inputs["att_dst2"], dtype=np.float32)
    b2 = np.asarray(inputs["b2"], dtype=np.float32)

    As = np.zeros((C1, HEADS), np.float32)
    Ad = np.zeros((C1, HEADS), np.float32)
    for h in range(HEADS):
        As[h * HID:(h + 1) * HID, h] = att_src1[h]
        Ad[h * HID:(h + 1) * HID, h] = att_dst1[h]
    # c-major column permutation for the h block: col c*8+h <- h*16+c
    perm = np.arange(C1).reshape(HEADS, HID).T.reshape(-1)
    W1cm = W1[:, perm]
    w1ext = np.concatenate([W1cm, W1 @ As, W1 @ Ad], axis=1).astype(np.float16)
    w2ext = np.concatenate(
        [W2, (W2 @ att_src2[0])[:, None], (W2 @ att_dst2[0])[:, None]],
        axis=1).astype(np.float16)

    x_pad = np.zeros((N_PAD, F_IN), np.float32)
    x_pad[:N_NODES] = x
    xT = np.ascontiguousarray(x_pad.T.astype(np.float16))

    b1b = np.tile(b1[None, :], (P, 1)).astype(np.float32)
    b2b = np.tile(b2[None, :], (P, 1)).astype(np.float32)
    ident = np.eye(P, dtype=np.float16)
    iota = np.ascontiguousarray(
        np.tile(np.arange(P, dtype=np.float16)[None, :], (P, 1)))

    in_maps = []
    for d in range(N_DEV):
        src_widx, dst_widx, dstl2 = dev[d]
        in_maps.append({
            "xT": xT, "w1ext": w1ext, "w2ext": w2ext, "b1b": b1b, "b2b": b2b,
            "ident": ident, "iota": iota,
            "swidx": src_widx, "dwidx": dst_widx, "dstli": dstl2,
            "xT_loc": np.ascontiguousarray(xT[:, d * NPD:(d + 1) * NPD]),
        })
    return in_maps


def kernel(x, edge_index, W1, att_src1, att_dst1, b1, W2, att_src2, att_dst2, b2):
    edge_index = np.asarray(edge_index, dtype=np.int32)
    plan, dev = _plan(edge_index)

    key = (tuple(plan["T1u"]), tuple(plan["T2u"]))
    if key not in _CACHE:
        _CACHE[key] = _build(plan)
    nc = _CACHE[key]

    in_maps = _make_in_maps(dict(
        x=x, W1=W1, att_src1=att_src1, att_dst1=att_dst1, b1=b1,
        W2=W2, att_src2=att_src2, att_dst2=att_dst2, b2=b2), plan, dev)
    res = run_bass_kernel_spmd(nc, in_maps, list(range(N_DEV)))
    full = np.concatenate([res.results[d]["out"] for d in range(N_DEV)], axis=0)
    return full[:N_NODES]


# revision 20
# speedup vs baseline: 1.0513x; 1.0045x over previous
"""Trainium2 Bass kernel for a 2-layer GAT (PyG semantics, eval mode).

SPMD over 8 NeuronCores, dst-sorted edge partitioning:
 - conv aggregation + softmax denominators fused into one-hot matmuls
   (psum += eq.T @ rhs) per 128-edge tile.
 - Per-edge gathers via batched Q7 dma_gather (1024 descs/call, the ucode
   ring limit) with cost-optimal element sizes: conv1 h rows 512B, conv1
   a_d 16B, conv2 rows 86B, conv2 a_d2 4B.
 - Source-row gather calls are stream-packed (cross node-tile boundaries,
   per-slot reads fragmented); a_d gathers are slot-windowed (in-tile
   indices, narrow table deps) and prefetched into persistent buffers so
   they overlap phase 0 (conv1) / conv1 compute (conv2).
 - conv2 uses weighted one-hots (fused is_equal*exp) and a constant-1
   table column: no per-edge rhs assembly, denominator from the same
   matmul.
 - h = x @ W1ext computed fully redundantly per core (c-major head
   layout); h2 AllGather in 4 chunks interleaved with conv1 compute.
"""
import sys

sys.path.insert(0, "/opt/trn_rl_repo")

import numpy as np

import concourse.bacc as bacc
import concourse.bass as bass
import concourse.mybir as mybir
import concourse.tile as tile
from concourse.bass_utils import run_bass_kernel_spmd

P = 128
N_DEV = 8
N_NODES = 50000
F_IN, HID, N_CLS, HEADS = 128, 16, 40, 8
NEG_SLOPE = 0.2
HALF = 32768                 # int16 index limit for dma_gather

N_TILES = 392
N_PAD = N_TILES * P          # 50176
TPD = N_TILES // N_DEV       # 49
NPD = TPD * P                # 6272

C1 = HEADS * HID             # 128 (h block, c-major: col c*8+h)
D1 = C1 + HEADS              # 136 rhs cols conv1: [h*ex | ex]
R1 = 256                     # conv1 table row elems (fp16, 512B stride)
RL = 128                     # conv2 table / ad_win row elems (256B stride)
D2 = N_CLS + 3               # 43 conv2 row: [h2lin(40)|a_s2|a_d2|one]
CMP = 44                     # compact collective row (43 used)
W2C = N_CLS + 2              # 42 = [W2 | w_as2 | w_ad2]

CT = 8                       # tiles per gather call (1024 descs: ring limit)
CB = 7                       # node tiles per post-processing chunk
NCOLL = 4                    # collective chunks
SLC = 8                      # src-idx calls per sidx load chunk

fp32 = mybir.dt.float32
fp16 = mybir.dt.float16
i16 = mybir.dt.int16

_CACHE = {}


def _patched_dma_gather():
    import inspect, textwrap
    src = inspect.getsource(bass.BassGpSimd.dma_gather)
    old = """        assert (
            elem_size_bytes > 0 and elem_size_bytes % 256 == 0
        )  # transpose restriction"""
    assert old in src, "bass dma_gather source changed; fall back to 256B elems"
    src = src.replace(old, """        assert elem_size_bytes > 0
        if transpose:
            assert elem_size_bytes % 256 == 0""")
    g = dict(bass.__dict__)
    exec(compile(textwrap.dedent(src), "<patched_dma_gather>", "exec"), g)
    return g["dma_gather"]


_PG = _patched_dma_gather()


def _wrap(idx_list):
    """int16 idx list -> [128, n/16] wrapped layout (j at [j%16, j//16],
    replicated across the 8 16-partition groups)."""
    n = len(idx_list)
    assert n % 16 == 0
    w = np.asarray(idx_list, dtype=np.int16).reshape(n // 16, 16).T
    return np.tile(w, (8, 1))


def _frags(cum, jj):
    """Split stream tiles [cum[jj], cum[jj+1]) at CT boundaries.
    Returns [(call, off_in_call, n, off_in_slot)]."""
    s0, s1 = int(cum[jj]), int(cum[jj + 1])
    res = []
    s = s0
    while s < s1:
        k = s // CT
        n = min((k + 1) * CT, s1) - s
        res.append((k, s - k * CT, n, s - s0))
        s += n
    return res


def _plan(edge_index):
    """Shared (cross-device) program plan + per-device data arrays."""
    src = np.concatenate([edge_index[0], np.arange(N_NODES, dtype=np.int32)])
    dst = np.concatenate([edge_index[1], np.arange(N_NODES, dtype=np.int32)])
    order = np.argsort(dst, kind="stable")
    src_s, dst_s = src[order], dst[order]
    bounds = np.searchsorted(dst_s, np.arange(N_TILES + 1) * P).astype(np.int64)

    lo_lists, hi_lists = [], []
    for j in range(N_TILES):
        s = src_s[bounds[j]:bounds[j + 1]]
        d = dst_s[bounds[j]:bounds[j + 1]]
        m = s < HALF
        lo_lists.append((s[m], d[m]))
        hi_lists.append((s[~m] - HALF, d[~m]))

    t1 = np.array([(len(lo_lists[j][0]) + P - 1) // P for j in range(N_TILES)])
    t2 = np.array([(len(hi_lists[j][0]) + P - 1) // P for j in range(N_TILES)])
    T1u = t1.reshape(N_DEV, TPD).max(axis=0)           # per-slot max
    T2u = t2.reshape(N_DEV, TPD).max(axis=0)
    Tu = T1u + T2u
    n_et = int(Tu.sum())

    lo_cum = np.concatenate([[0], np.cumsum(T1u)]).astype(int)
    hi_cum = np.concatenate([[0], np.cumsum(T2u)]).astype(int)
    d_cum = np.concatenate([[0], np.cumsum(Tu)]).astype(int)
    LOT, HIT = int(lo_cum[-1]), int(hi_cum[-1])
    n_lo_calls = (LOT + CT - 1) // CT
    n_hi_calls = (HIT + CT - 1) // CT

    plan = dict(T1u=T1u, T2u=T2u, Tu=Tu, n_et=n_et,
                lo_cum=lo_cum, hi_cum=hi_cum, d_cum=d_cum,
                LOT=LOT, HIT=HIT, n_lo_calls=n_lo_calls, n_hi_calls=n_hi_calls)

    dev = []
    for d in range(N_DEV):
        lo_stream = np.zeros(n_lo_calls * CT * P, np.int32)
        hi_stream = np.zeros(n_hi_calls * CT * P, np.int32)
        dit = np.zeros(n_et * P, np.int32)      # in-tile dst idx (0..127)
        dstl = np.full(n_et * P, 300.0, np.float32)
        for jj in range(TPD):
            j = d * TPD + jj
            sl, dl = lo_lists[j]
            sh, dh = hi_lists[j]
            lo_stream[lo_cum[jj] * P:lo_cum[jj] * P + len(sl)] = sl
            hi_stream[hi_cum[jj] * P:hi_cum[jj] * P + len(sh)] = sh
            base = d_cum[jj] * P
            dit[base:base + len(dl)] = dl - j * P
            dit[base + T1u[jj] * P:base + T1u[jj] * P + len(dh)] = dh - j * P
            dstl[base:base + len(dl)] = dl - j * P
            dstl[base + T1u[jj] * P:base + T1u[jj] * P + len(dh)] = dh - j * P
        sblocks = []
        for k in range(n_lo_calls):
            sblocks.append(_wrap(lo_stream[k * CT * P:(k + 1) * CT * P]))
        for k in range(n_hi_calls):
            sblocks.append(_wrap(hi_stream[k * CT * P:(k + 1) * CT * P]))
        src_widx = np.ascontiguousarray(
            np.concatenate(sblocks, axis=1)).astype(np.int16)
        dblocks = []
        for jj in range(TPD):
            T = int(Tu[jj])
            base = d_cum[jj] * P
            for c0 in range(0, T, CT):
                n = min(CT, T - c0)
                dblocks.append(_wrap(dit[base + c0 * P:base + (c0 + n) * P]))
        dst_widx = np.ascontiguousarray(
            np.concatenate(dblocks, axis=1)).astype(np.int16)
        dstl2 = np.ascontiguousarray(dstl.reshape(n_et, P).T)
        dev.append((src_widx, dst_widx, dstl2))
    return plan, dev


def _build(plan):
    T1u, T2u, Tu = plan["T1u"], plan["T2u"], plan["Tu"]
    n_et = plan["n_et"]
    lo_cum, hi_cum, d_cum = plan["lo_cum"], plan["hi_cum"], plan["d_cum"]
    LOT, HIT = plan["LOT"], plan["HIT"]
    n_lo_calls, n_hi_calls = plan["n_lo_calls"], plan["n_hi_calls"]
    Tmax = int(Tu.max())

    lo_col = lambda k: k * CT * 8
    hi_col = lambda k: (n_lo_calls + k) * CT * 8
    SRC_COLS = (n_lo_calls + n_hi_calls) * CT * 8
    dw_col = {}
    c = 0
    for jj in range(TPD):
        T = int(Tu[jj])
        for c0 in range(0, T, CT):
            n = min(CT, T - c0)
            dw_col[(jj, c0)] = c
            c += n * 8
    DST_COLS = c

    cbnd = [0, 14, 28, 42, TPD]
    coll_after_chunk = {2: [0], 4: [1], 5: [2], 6: [3]}

    nc = bacc.Bacc("TRN2", target_bir_lowering=False, debug=False,
                   num_devices=N_DEV, num_swdge_queues=4)

    xT = nc.dram_tensor("xT", [P, N_PAD], fp16, kind="ExternalInput")
    xT_loc = nc.dram_tensor("xT_loc", [P, NPD], fp16, kind="ExternalInput")
    w1ext = nc.dram_tensor("w1ext", [P, D1 + HEADS], fp16, kind="ExternalInput")
    w2ext = nc.dram_tensor("w2ext", [HID, W2C], fp16, kind="ExternalInput")
    b1b = nc.dram_tensor("b1b", [P, HID], fp32, kind="ExternalInput")
    b2b = nc.dram_tensor("b2b", [P, N_CLS], fp32, kind="ExternalInput")
    ident = nc.dram_tensor("ident", [P, P], fp16, kind="ExternalInput")
    iota = nc.dram_tensor("iota", [P, P], fp16, kind="ExternalInput")
    swidx = nc.dram_tensor("swidx", [P, SRC_COLS], i16, kind="ExternalInput")
    dwidx = nc.dram_tensor("dwidx", [P, DST_COLS], i16, kind="ExternalInput")
    dstli = nc.dram_tensor("dstli", [P, n_et], fp32, kind="ExternalInput")
    out = nc.dram_tensor("out", [NPD, N_CLS], fp32, kind="ExternalOutput")

    h_lo_t = nc.dram_tensor("h_lo_t", [HALF, R1], fp16)
    h_hi_t = nc.dram_tensor("h_hi_t", [N_PAD - HALF, R1], fp16)
    ad_win = nc.dram_tensor("ad_win", [NPD, RL], fp16)
    h2_loc = nc.dram_tensor("h2_loc", [NPD, RL], fp16)
    h2_cmp_loc = nc.dram_tensor("h2_cmp_loc", [NPD, CMP], fp16)
    h2_gath = nc.dram_tensor("h2_gath", [N_PAD, CMP], fp16, addr_space="Shared")
    h2_ext = nc.dram_tensor("h2_ext", [N_PAD, RL], fp16)

    rg = [list(range(N_DEV))]
    AO = mybir.AluOpType
    AF = mybir.ActivationFunctionType
    AX = mybir.AxisListType.X
    _q = [0]

    def qn():
        _q[0] = (_q[0] + 1) % 4
        return _q[0]

    with tile.TileContext(nc) as tc:
        with (
            tc.tile_pool(name="const", bufs=1) as cpool,
            tc.tile_pool(name="glo", bufs=8) as glo,
            tc.tile_pool(name="ghi", bufs=8) as ghi,
            tc.tile_pool(name="ph0", bufs=2) as ph0,
            tc.tile_pool(name="sidx", bufs=2) as sxp,
            tc.tile_pool(name="work", bufs=3) as wp,
            tc.tile_pool(name="eqp", bufs=32) as eqp,
            tc.tile_pool(name="small", bufs=2) as sm,
            tc.tile_pool(name="psA", bufs=3, space="PSUM") as psA,
            tc.tile_pool(name="psB", bufs=3, space="PSUM") as psB,
            tc.tile_pool(name="psC", bufs=2, space="PSUM") as psC,
        ):
            # ---------------- constants
            w1_sb = cpool.tile([P, D1 + HEADS], fp16)
            nc.sync.dma_start(out=w1_sb[:], in_=w1ext[:])
            w2_sb = cpool.tile([HID, W2C], fp16)
            nc.sync.dma_start(out=w2_sb[:], in_=w2ext[:])
            b1_sb = cpool.tile([P, HID], fp32)
            nc.sync.dma_start(out=b1_sb[:], in_=b1b[:])
            b2_sb = cpool.tile([P, N_CLS], fp32)
            nc.sync.dma_start(out=b2_sb[:], in_=b2b[:])
            id_sb = cpool.tile([P, P], fp16)
            nc.sync.dma_start(out=id_sb[:], in_=ident[:])
            iota_sb = cpool.tile([P, P], fp16)
            nc.sync.dma_start(out=iota_sb[:], in_=iota[:])
            dwidx_sb = cpool.tile([P, DST_COLS], i16)
            nc.sync.dma_start(out=dwidx_sb[:], in_=dwidx[:])
            dstl_sb = cpool.tile([P, n_et], fp32)
            nc.sync.dma_start(out=dstl_sb[:], in_=dstli[:])
            ones_sb = cpool.tile([P, 1], fp16)
            nc.vector.memset(ones_sb[:], 1.0)
            h1all = cpool.tile([P, TPD * HID], fp16)
            ad1c = cpool.tile([P, n_et * HEADS], fp16)   # conv1 a_d per edge
            ad2c = cpool.tile([P, n_et * 2], fp16)       # conv2 a_d2 per edge

            # ---------------- ad_win: local a_d rows from xT_loc
            hst2 = cpool.tile([P, TPD * HEADS], fp16)
            NC2 = 7
            for cc in range(TPD // NC2):
                xc2 = ph0.tile([P, NC2 * P], fp16, tag="xc")
                nc.sync.dma_start(
                    out=xc2[:], in_=xT_loc[:, cc * NC2 * P:(cc + 1) * NC2 * P])
                psd = psB.tile([P, NC2 * HEADS], fp32, tag="acc", space="PSUM")
                for k in range(NC2):
                    nc.tensor.matmul(out=psd[:, k * HEADS:(k + 1) * HEADS],
                                     lhsT=xc2[:, k * P:(k + 1) * P],
                                     rhs=w1_sb[:, D1:D1 + HEADS],
                                     start=True, stop=True)
                nc.scalar.copy(out=hst2[:, cc * NC2 * HEADS:(cc + 1) * NC2 * HEADS],
                               in_=psd[:])
            nc.sync.dma_start(
                out=ad_win[:, 0:HEADS].rearrange("(t p) d -> p t d", p=P),
                in_=hst2[:].rearrange("p (t d) -> p t d", d=HEADS))

            # conv1 a_d prefetch calls: emitted interleaved with phase 0
            ad1_calls = [(jj, c0) for jj in range(TPD)
                         for c0 in range(0, int(Tu[jj]), CT)]
            ad1_pos = [0]

            def emit_ad1(k):
                for (jj, c0) in ad1_calls[ad1_pos[0]:ad1_pos[0] + k]:
                    n = min(CT, int(Tu[jj]) - c0)
                    dc = dw_col[(jj, c0)]
                    _PG(nc.gpsimd,
                        out_ap=ad1c[:].rearrange("p (t d) -> p t d", d=HEADS)[
                            :, int(d_cum[jj]) + c0:int(d_cum[jj]) + c0 + n, :],
                        in_ap=ad_win[jj * P:(jj + 1) * P, 0:HEADS],
                        idxs_ap=dwidx_sb[:, dc:dc + n * 8],
                        num_idxs=n * P, num_idxs_reg=n * P,
                        elem_size=HEADS, elem_step=RL, queue_num=qn())
                ad1_pos[0] += k

            # ---------------- phase 0: full-redundant h table (c-major rows)
            NCHUNK = 8
            for cc in range(N_TILES // NCHUNK):
                xc = ph0.tile([P, NCHUNK * P], fp16, tag="xc")
                nc.sync.dma_start(
                    out=xc[:], in_=xT[:, cc * NCHUNK * P:(cc + 1) * NCHUNK * P])
                hst = ph0.tile([P, NCHUNK * R1], fp16, tag="hst")
                for gi, (g0, gn) in enumerate(((0, 3), (3, 3), (6, 2))):
                    psh = psA.tile([P, 3 * (D1 + HEADS)], fp32, tag="big",
                                   space="PSUM")
                    for k in range(gn):
                        nc.tensor.matmul(
                            out=psh[:, k * (D1 + HEADS):(k + 1) * (D1 + HEADS)],
                            lhsT=xc[:, (g0 + k) * P:(g0 + k + 1) * P],
                            rhs=w1_sb[:], start=True, stop=True)
                    dst_view = hst[:].rearrange("p (k d) -> p k d", d=R1)[
                        :, g0:g0 + gn, 0:D1 + HEADS]
                    src_view = psh[:].rearrange("p (k d) -> p k d",
                                                d=D1 + HEADS)[:, 0:gn, :]
                    if gi % 2 == 0:
                        nc.scalar.copy(out=dst_view, in_=src_view)
                    else:
                        nc.vector.tensor_copy(out=dst_view, in_=src_view)
                r0 = cc * NCHUNK * P
                tgt = (h_lo_t[r0:r0 + NCHUNK * P, :] if r0 < HALF
                       else h_hi_t[r0 - HALF:r0 - HALF + NCHUNK * P, :])
                nc.sync.dma_start(
                    out=tgt.rearrange("(k p) d -> p k d", p=P),
                    in_=hst[:].rearrange("p (k d) -> p k d", d=R1))
                emit_ad1(4)

            emit_ad1(len(ad1_calls) - ad1_pos[0])

            # ---- conv1 post: ELU(mean(agg/den) + b1) -> h1all, then h2 rows
            def post1(acc, jj0, nb):
                a_v = acc[:].rearrange("p (b d) -> p b d", d=D1)
                den = sm.tile([P, CB * HEADS], fp32, tag="den")
                nc.vector.tensor_scalar(
                    out=den[:].rearrange("p (b h) -> p b h", h=HEADS)[:, 0:nb, :],
                    in0=a_v[:, 0:nb, C1:D1], scalar1=1e-16, scalar2=None,
                    op0=AO.add)
                rec = sm.tile([P, CB * HEADS], fp32, tag="rec")
                nc.vector.reciprocal(out=rec[:, 0:nb * HEADS],
                                     in_=den[:, 0:nb * HEADS])
                nc.vector.tensor_scalar(out=rec[:, 0:nb * HEADS],
                                        in0=rec[:, 0:nb * HEADS],
                                        scalar1=1.0 / HEADS, scalar2=None,
                                        op0=AO.mult)
                tmp = sm.tile([P, CB * C1], fp32, tag="tmp")
                nc.vector.tensor_tensor(
                    out=tmp[:].rearrange("p (b c h) -> p b c h",
                                         c=HID, h=HEADS)[:, 0:nb],
                    in0=a_v[:, 0:nb, 0:C1].rearrange("p b (c h) -> p b c h",
                                                     h=HEADS),
                    in1=rec[:].rearrange("p (b h) -> p b h", h=HEADS)[:, 0:nb, :]
                        .unsqueeze(2).to_broadcast([P, nb, HID, HEADS]),
                    op=AO.mult)
                h1b = sm.tile([P, CB * HID], fp32, tag="h1b")
                nc.vector.tensor_reduce(
                    out=h1b[:].rearrange("p (b c) -> p b c", c=HID)[:, 0:nb, :],
                    in_=tmp[:].rearrange("p (b c h) -> p b c h",
                                         c=HID, h=HEADS)[:, 0:nb],
                    axis=AX, op=AO.add)
                nc.vector.tensor_tensor(
                    out=h1b[:].rearrange("p (b c) -> p b c", c=HID)[:, 0:nb, :],
                    in0=h1b[:].rearrange("p (b c) -> p b c", c=HID)[:, 0:nb, :],
                    in1=b1_sb[:].unsqueeze(1).to_broadcast([P, nb, HID]),
                    op=AO.add)
                xm = sm.tile([P, CB * HID], fp32, tag="xm")
                nc.vector.tensor_scalar(out=xm[:, 0:nb * HID],
                                        in0=h1b[:, 0:nb * HID],
                                        scalar1=0.0, scalar2=None, op0=AO.min)
                em = sm.tile([P, CB * HID], fp32, tag="em")
                nc.scalar.activation(out=em[:, 0:nb * HID], in_=xm[:, 0:nb * HID],
                                     func=AF.Exp)
                xp = sm.tile([P, CB * HID], fp32, tag="xp")
                nc.vector.tensor_scalar(out=xp[:, 0:nb * HID],
                                        in0=h1b[:, 0:nb * HID],
                                        scalar1=0.0, scalar2=None, op0=AO.max)
                h1f = sm.tile([P, CB * HID], fp32, tag="h1f")
                nc.vector.tensor_tensor(out=h1f[:, 0:nb * HID],
                                        in0=em[:, 0:nb * HID],
                                        in1=xp[:, 0:nb * HID], op=AO.add)
                nc.vector.tensor_scalar(out=h1all[:, jj0 * HID:(jj0 + nb) * HID],
                                        in0=h1f[:, 0:nb * HID],
                                        scalar1=-1.0, scalar2=None, op0=AO.add)
                for i in range(nb):
                    jj = jj0 + i
                    pst = psC.tile([HID, P], fp16, tag="tp", space="PSUM")
                    nc.tensor.transpose(out=pst[:],
                                        in_=h1all[:, jj * HID:(jj + 1) * HID],
                                        identity=id_sb[:])
                    h1T = sm.tile([HID, P], fp16, tag="h1T")
                    nc.scalar.copy(out=h1T[:], in_=pst[:])
                    psh2 = psC.tile([P, W2C], fp32, tag="tp", space="PSUM")
                    nc.tensor.matmul(out=psh2[:], lhsT=h1T[:], rhs=w2_sb[:],
                                     start=True, stop=True)
                    h2st = sm.tile([P, CMP], fp16, tag="h2st")
                    nc.scalar.copy(out=h2st[:, 0:W2C], in_=psh2[:])
                    nc.vector.tensor_copy(out=h2st[:, W2C:W2C + 1], in_=ones_sb[:])
                    nc.sync.dma_start(out=h2_loc[jj * P:(jj + 1) * P, 0:CMP],
                                      in_=h2st[:])
                    nc.sync.dma_start(out=h2_cmp_loc[jj * P:(jj + 1) * P, :],
                                      in_=h2st[:])
                # conv2 a_d2 gathers for these tiles (overlap conv1/collective)
                for i in range(nb):
                    jj = jj0 + i
                    T = int(Tu[jj])
                    for c0 in range(0, T, CT):
                        n = min(CT, T - c0)
                        dc = dw_col[(jj, c0)]
                        _PG(nc.gpsimd,
                            out_ap=ad2c[:].rearrange("p (t d) -> p t d", d=2)[
                                :, int(d_cum[jj]) + c0:int(d_cum[jj]) + c0 + n, :],
                            in_ap=h2_loc[jj * P:(jj + 1) * P,
                                         N_CLS + 1:N_CLS + 3],
                            idxs_ap=dwidx_sb[:, dc:dc + n * 8],
                            num_idxs=n * P, num_idxs_reg=n * P,
                            elem_size=2, elem_step=RL, queue_num=qn())

            # ---- conv2 post: log_softmax(agg/den + b2) -> out
            def post2(acc, jj0, nb):
                a_v = acc[:].rearrange("p (b d) -> p b d", d=D2)
                den = sm.tile([P, CB], fp32, tag="den2")
                nc.vector.tensor_scalar(
                    out=den[:].rearrange("p (b o) -> p b o", o=1)[:, 0:nb, :],
                    in0=a_v[:, 0:nb, D2 - 1:D2], scalar1=1e-16, scalar2=None,
                    op0=AO.add)
                rec = sm.tile([P, CB], fp32, tag="rec2")
                nc.vector.reciprocal(out=rec[:, 0:nb], in_=den[:, 0:nb])
                h2f = sm.tile([P, CB * N_CLS], fp32, tag="h2f")
                h2f_v = h2f[:].rearrange("p (b c) -> p b c", c=N_CLS)
                nc.vector.tensor_tensor(
                    out=h2f_v[:, 0:nb, :], in0=a_v[:, 0:nb, 0:N_CLS],
                    in1=rec[:].rearrange("p (b o) -> p b o", o=1)[:, 0:nb, :]
                        .to_broadcast([P, nb, N_CLS]),
                    op=AO.mult)
                nc.vector.tensor_tensor(
                    out=h2f_v[:, 0:nb, :], in0=h2f_v[:, 0:nb, :],
                    in1=b2_sb[:].unsqueeze(1).to_broadcast([P, nb, N_CLS]),
                    op=AO.add)
                nm = sm.tile([P, CB], fp32, tag="nm")
                nc.vector.tensor_reduce(
                    out=nm[:].rearrange("p (b o) -> p b o", o=1)[:, 0:nb, :],
                    in_=h2f_v[:, 0:nb, :], axis=AX, op=AO.max, negate=True)
                hs = sm.tile([P, CB * N_CLS], fp32, tag="hs")
                hs_v = hs[:].rearrange("p (b c) -> p b c", c=N_CLS)
                nc.vector.tensor_tensor(
                    out=hs_v[:, 0:nb, :], in0=h2f_v[:, 0:nb, :],
                    in1=nm[:].rearrange("p (b o) -> p b o", o=1)[:, 0:nb, :]
                        .to_broadcast([P, nb, N_CLS]),
                    op=AO.add)
                es = sm.tile([P, CB * N_CLS], fp32, tag="es")
                nc.scalar.activation(out=es[:, 0:nb * N_CLS],
                                     in_=hs[:, 0:nb * N_CLS], func=AF.Exp)
                ssum = sm.tile([P, CB], fp32, tag="ssum")
                nc.vector.tensor_reduce(
                    out=ssum[:].rearrange("p (b o) -> p b o", o=1)[:, 0:nb, :],
                    in_=es[:].rearrange("p (b c) -> p b c", c=N_CLS)[:, 0:nb, :],
                    axis=AX, op=AO.add)
                lg = sm.tile([P, CB], fp32, tag="lg")
                nc.scalar.activation(out=lg[:, 0:nb], in_=ssum[:, 0:nb],
                                     func=AF.Ln)
                ot = sm.tile([P, CB * N_CLS], fp32, tag="ot")
                nc.vector.tensor_tensor(
                    out=ot[:].rearrange("p (b c) -> p b c", c=N_CLS)[:, 0:nb, :],
                    in0=hs_v[:, 0:nb, :],
                    in1=lg[:].rearrange("p (b o) -> p b o", o=1)[:, 0:nb, :]
                        .to_broadcast([P, nb, N_CLS]),
                    op=AO.subtract)
                nc.sync.dma_start(
                    out=out[jj0 * P:(jj0 + nb) * P, :]
                        .rearrange("(b p) d -> p b d", p=P),
                    in_=ot[:].rearrange("p (b c) -> p b c", c=N_CLS)[:, 0:nb, :])

            def emit_coll(q):
                r0, r1 = cbnd[q] * P, cbnd[q + 1] * P
                nc.gpsimd.collective_compute(
                    "AllGather", AO.bypass, replica_groups=rg,
                    ins=[h2_cmp_loc[r0:r1, :].opt()],
                    outs=[h2_gath[r0 * N_DEV:r1 * N_DEV, :].opt()])

            def emit_expand(q):
                r0, r1 = cbnd[q] * P, cbnd[q + 1] * P
                nc.sync.dma_start(
                    out=h2_ext[:, 0:CMP]
                        .rearrange("(d r) c -> d r c", d=N_DEV)[:, r0:r1, :],
                    in_=h2_gath[r0 * N_DEV:r1 * N_DEV, :]
                        .rearrange("(d r) c -> d r c", d=N_DEV, r=r1 - r0))

            # ---------------- shared conv loop
            def conv_pass(conv):
                RW = R1 if conv == 1 else D2
                lo_tiles = {}
                hi_tiles = {}
                next_lo = next_hi = 0
                state = {"lo": [None, 0, -1], "hi": [None, 0, -1]}  # tile,c0,c1

                def load_sidx(st, col0):
                    col1 = min(col0 + SLC * CT * 8, SRC_COLS)
                    t = sxp.tile([P, SLC * CT * 8], i16, tag="sidx_" + st)
                    nc.sync.dma_start(out=t[:, 0:col1 - col0],
                                      in_=swidx[:, col0:col1])
                    state[st] = [t, col0, col1]

                def idx_view(st, c0, ncols):
                    t, s0, s1 = state[st]
                    if t is None or c0 < s0 or c0 + ncols > s1:
                        load_sidx(st, c0)
                        t, s0, s1 = state[st]
                    return t[:, c0 - s0:c0 - s0 + ncols]

                def emit_lo(k):
                    nonlocal next_lo
                    nt = min(CT, LOT - k * CT)
                    iv = idx_view("lo", lo_col(k), nt * 8)
                    t = glo.tile([P, CT * RW], fp16,
                                 tag="rlo" if conv == 1 else "rlo2")
                    v = t[:].rearrange("p (t d) -> p t d", d=RW)
                    if conv == 1:
                        _PG(nc.gpsimd, out_ap=v[:, 0:nt, :], in_ap=h_lo_t[:, :],
                            idxs_ap=iv, num_idxs=nt * P, num_idxs_reg=nt * P,
                            elem_size=R1, queue_num=qn())
                    else:
                        _PG(nc.gpsimd, out_ap=v[:, 0:nt, :],
                            in_ap=h2_ext[0:HALF, 0:D2],
                            idxs_ap=iv, num_idxs=nt * P, num_idxs_reg=nt * P,
                            elem_size=D2, elem_step=RL, queue_num=qn())
                    lo_tiles[k] = v
                    next_lo = k + 1

                def emit_hi(k):
                    nonlocal next_hi
                    nt = min(CT, HIT - k * CT)
                    iv = idx_view("hi", hi_col(k), nt * 8)
                    t = ghi.tile([P, CT * RW], fp16,
                                 tag="rhi" if conv == 1 else "rhi2")
                    v = t[:].rearrange("p (t d) -> p t d", d=RW)
                    if conv == 1:
                        _PG(nc.gpsimd, out_ap=v[:, 0:nt, :], in_ap=h_hi_t[:, :],
                            idxs_ap=iv, num_idxs=nt * P, num_idxs_reg=nt * P,
                            elem_size=R1, queue_num=qn())
                    else:
                        _PG(nc.gpsimd, out_ap=v[:, 0:nt, :],
                            in_ap=h2_ext[HALF:N_PAD, 0:D2],
                            idxs_ap=iv, num_idxs=nt * P, num_idxs_reg=nt * P,
                            elem_size=D2, elem_step=RL, queue_num=qn())
                    hi_tiles[k] = v
                    next_hi = k + 1

                acc = None
                jj0 = 0
                for jj in range(TPD):
                    T1, T2, T = int(T1u[jj]), int(T2u[jj]), int(Tu[jj])
                    lofr = _frags(lo_cum, jj)
                    hifr = _frags(hi_cum, jj)
                    while next_lo * CT < lo_cum[jj + 1]:
                        emit_lo(next_lo)
                    while next_hi * CT < hi_cum[jj + 1]:
                        emit_hi(next_hi)

                    if jj % CB == 0:
                        jj0 = jj
                        acc = sm.tile([P, CB * (D1 if conv == 1 else D2)], fp32,
                                      tag="acc1" if conv == 1 else "acc2")

                    db = int(d_cum[jj])
                    if conv == 1:
                        ad_v = ad1c[:].rearrange("p (t d) -> p t d", d=HEADS)
                        e_t = sm.tile([P, Tmax * HEADS], fp16, tag="e")
                        e_v = e_t[:].rearrange("p (t h) -> p t h", h=HEADS)
                        for (k, co, n, so) in lofr:
                            nc.vector.tensor_tensor(
                                out=e_v[:, so:so + n, :],
                                in0=lo_tiles[k][:, co:co + n, C1:C1 + HEADS],
                                in1=ad_v[:, db + so:db + so + n, :], op=AO.add)
                        for (k, co, n, so) in hifr:
                            nc.vector.tensor_tensor(
                                out=e_v[:, T1 + so:T1 + so + n, :],
                                in0=hi_tiles[k][:, co:co + n, C1:C1 + HEADS],
                                in1=ad_v[:, db + T1 + so:db + T1 + so + n, :],
                                op=AO.add)
                        e2_t = sm.tile([P, Tmax * HEADS], fp16, tag="e2")
                        nc.vector.scalar_tensor_tensor(
                            out=e2_t[:, 0:T * HEADS], in0=e_t[:, 0:T * HEADS],
                            scalar=NEG_SLOPE, in1=e_t[:, 0:T * HEADS],
                            op0=AO.mult, op1=AO.max)
                        ex_t = sm.tile([P, Tmax * HEADS], fp16, tag="ex")
                        nc.scalar.activation(out=ex_t[:, 0:T * HEADS],
                                             in_=e2_t[:, 0:T * HEADS],
                                             func=AF.Exp)
                        ex_v = ex_t[:].rearrange("p (t h) -> p t h", h=HEADS)
                        rhs = wp.tile([P, Tmax * D1], fp16, tag="rhs")
                        rhs_v = rhs[:].rearrange("p (t d) -> p t d", d=D1)
                        nc.vector.tensor_copy(out=rhs_v[:, 0:T, C1:D1],
                                              in_=ex_v[:, 0:T, :])
                        for (k, co, n, so) in lofr:
                            nc.vector.tensor_tensor(
                                out=rhs_v[:, so:so + n, 0:C1].rearrange(
                                    "p t (c h) -> p t c h", h=HEADS),
                                in0=lo_tiles[k][:, co:co + n, 0:C1].rearrange(
                                    "p t (c h) -> p t c h", h=HEADS),
                                in1=ex_v[:, so:so + n, :].unsqueeze(2)
                                    .to_broadcast([P, n, HID, HEADS]),
                                op=AO.mult)
                        for (k, co, n, so) in hifr:
                            nc.vector.tensor_tensor(
                                out=rhs_v[:, T1 + so:T1 + so + n, 0:C1].rearrange(
                                    "p t (c h) -> p t c h", h=HEADS),
                                in0=hi_tiles[k][:, co:co + n, 0:C1].rearrange(
                                    "p t (c h) -> p t c h", h=HEADS),
                                in1=ex_v[:, T1 + so:T1 + so + n, :].unsqueeze(2)
                                    .to_broadcast([P, n, HID, HEADS]),
                                op=AO.mult)
                        ps1 = psB.tile([P, D1], fp32, tag="acc", space="PSUM")
                        for t in range(T):
                            eq = eqp.tile([P, P], fp16, tag="eq")
                            nc.vector.tensor_scalar(
                                out=eq[:], in0=iota_sb[:],
                                scalar1=dstl_sb[:, db + t:db + t + 1],
                                scalar2=None, op0=AO.is_equal)
                            nc.tensor.matmul(out=ps1[:], lhsT=eq[:],
                                             rhs=rhs[:, t * D1:(t + 1) * D1],
                                             start=(t == 0), stop=(t == T - 1))
                        nc.scalar.copy(
                            out=acc[:, (jj - jj0) * D1:(jj - jj0 + 1) * D1],
                            in_=ps1[:])
                    else:
                        ad_v = ad2c[:].rearrange("p (t d) -> p t d", d=2)
                        e_t = sm.tile([P, Tmax], fp16, tag="ec")
                        e_v = e_t[:].rearrange("p (t o) -> p t o", o=1)
                        for (k, co, n, so) in lofr:
                            nc.vector.tensor_tensor(
                                out=e_v[:, so:so + n, :],
                                in0=lo_tiles[k][:, co:co + n, N_CLS:N_CLS + 1],
                                in1=ad_v[:, db + so:db + so + n, 0:1], op=AO.add)
                        for (k, co, n, so) in hifr:
                            nc.vector.tensor_tensor(
                                out=e_v[:, T1 + so:T1 + so + n, :],
                                in0=hi_tiles[k][:, co:co + n, N_CLS:N_CLS + 1],
                                in1=ad_v[:, db + T1 + so:db + T1 + so + n, 0:1],
                                op=AO.add)
                        e2_t = sm.tile([P, Tmax], fp16, tag="e2c")
                        nc.vector.scalar_tensor_tensor(
                            out=e2_t[:, 0:T], in0=e_t[:, 0:T], scalar=NEG_SLOPE,
                            in1=e_t[:, 0:T], op0=AO.mult, op1=AO.max)
                        ex2_t = sm.tile([P, Tmax], fp32, tag="ex2")
                        nc.scalar.activation(out=ex2_t[:, 0:T], in_=e2_t[:, 0:T],
                                             func=AF.Exp)
                        ps2 = psB.tile([P, D2], fp32, tag="acc", space="PSUM")
                        t = 0
                        for (k, co, n, so) in lofr + hifr:
                            tl = lo_tiles if t < T1 else hi_tiles
                            for i in range(n):
                                eqw = eqp.tile([P, P], fp16, tag="eq")
                                nc.vector.tensor_scalar(
                                    out=eqw[:], in0=iota_sb[:],
                                    scalar1=dstl_sb[:, db + t:db + t + 1],
                                    scalar2=ex2_t[:, t:t + 1],
                                    op0=AO.is_equal, op1=AO.mult)
                                nc.tensor.matmul(
                                    out=ps2[:], lhsT=eqw[:],
                                    rhs=tl[k][:, co + i, :],
                                    start=(t == 0), stop=(t == T - 1))
                                t += 1
                        nc.scalar.copy(
                            out=acc[:, (jj - jj0) * D2:(jj - jj0 + 1) * D2],
                            in_=ps2[:])

                    if jj - jj0 + 1 == CB or jj == TPD - 1:
                        nb = jj - jj0 + 1
                        if conv == 1:
                            post1(acc, jj0, nb)
                            ck = jj // CB
                            for q in coll_after_chunk.get(ck, []):
                                emit_coll(q)
                        else:
                            post2(acc, jj0, nb)

            # ---------------- conv1 (collectives + a_d2 gathers interleaved)
            conv_pass(1)
            for q in range(NCOLL):
                emit_expand(q)
            # ---------------- conv2
            conv_pass(2)

    nc.compile()
    return nc


def _make_in_maps(inputs, plan, dev):
    x = np.asarray(inputs["x"], dtype=np.float32)
    W1 = np.asarray(inputs["W1"], dtype=np.float32)
    att_src1 = np.asarray(inputs["att_src1"], dtype=np.float32)
    att_dst1 = np.asarray(inputs["att_dst1"], dtype=np.float32)
    b1 = np.asarray(inputs["b1"], dtype=np.float32)
    W2 = np.asarray(inputs["W2"], dtype=np.float32)
    att_src2 = np.asarray(inputs["att_src2"], dtype=np.float32)
    att_dst2 = np.asarray(inputs["att_dst2"], dtype=np.float32)
    b2 = np.asarray(inputs["b2"], dtype=np.float32)

    As = np.zeros((C1, HEADS), np.float32)
    Ad = np.zeros((C1, HEADS), np.float32)
    for h in range(HEADS):
        As[h * HID:(h + 1) * HID, h] = att_src1[h]
        Ad[h * HID:(h + 1) * HID, h] = att_dst1[h]
    # c-major column permutation for the h block: col c*8+h <- h*16+c
    perm = np.arange(C1).reshape(HEADS, HID).T.reshape(-1)
    W1cm = W1[:, perm]
    w1ext = np.concatenate([W1cm, W1 @ As, W1 @ Ad], axis=1).astype(np.float16)
    w2ext = np.concatenate(
        [W2, (W2 @ att_src2[0])[:, None], (W2 @ att_dst2[0])[:, None]],
        axis=1).astype(np.float16)

    x_pad = np.zeros((N_PAD, F_IN), np.float32)
    x_pad[:N_NODES] = x
    xT = np.ascontiguousarray(x_pad.T.astype(np.float16))

    b1b = np.tile(b1[None, :], (P, 1)).astype(np.float32)
    b2b = np.tile(b2[None, :], (P, 1)).astype(np.float32)
    ident = np.eye(P, dtype=np.float16)
    iota = np.ascontiguousarray(
        np.tile(np.arange(P, dtype=np.float16)[None, :], (P, 1)))

    in_maps = []
    for d in range(N_DEV):
        src_widx, dst_widx, dstl2 = dev[d]
        in_maps.append({
            "xT": xT, "w1ext": w1ext, "w2ext": w2ext, "b1b": b1b, "b2b": b2b,
            "ident": ident, "iota": iota,
            "swidx": src_widx, "dwidx": dst_widx, "dstli": dstl2,
            "xT_loc": np.ascontiguousarray(xT[:, d * NPD:(d + 1) * NPD]),
        })
    return in_maps


def kernel(x, edge_index, W1, att_src1, att_dst1, b1, W2, att_src2, att_dst2, b2):
    edge_index = np.asarray(edge_index, dtype=np.int32)
    plan, dev = _plan(edge_index)

    key = (tuple(plan["T1u"]), tuple(plan["T2u"]))
    if key not in _CACHE:
        _CACHE[key] = _build(plan)
    nc = _CACHE[key]

    in_maps = _make_in_maps(dict(
        x=x, W1=W1, att_src1=att_src1, att_dst1=att_dst1, b1=b1,
        W2=W2, att_src2=att_src2, att_dst2=att_dst2, b2=b2), plan, dev)
    res = run_bass_kernel_spmd(nc, in_maps, list(range(N_DEV)))
    full = np.concatenate([res.results[d]["out"] for d in range(N_DEV)], axis=0)
    return full[:N_NODES]
